# revision 18
# baseline (speedup 1.0000x reference)
"""Trainium2 Bass kernel for nn_BaseMOE (moe_routing), 8 NeuronCores.

Batch-sharded (B=256 -> 32 rows/core); full inputs in, full output out.

Per core:
  * 3-layer MLP + Wout on its [16 experts x 32 batch] rows in bf16.
    LayerNorm affine folded into the next layer's weights on the host;
    ELU via h = max(z, min(exp(z)-1, 0)); PSUM released early through an
    Activation-engine copy; LN sqrt/reciprocal on DVE so the Activation
    engine never swaps function tables; dummy matmuls warm the PE
    p-state before the MLP and through the collective gap.
  * softmax-over-batch: local exp(scores), per-expert partial sums
    exchanged with a 64-byte AllGather; all post-collective weight prep
    runs on Pool so the DVE queue (busy with threshold planes) never
    blocks on it.
  * scatter: idx[e,b,k] = 12*k + offs, offs in [0,12).  DVE builds 11
    *threshold* planes D_t = p * 1[offs < t] per [128,2048] tile with
    single tensor_mask ops (2x mode); the raw probs tile is the 12th
    plane.  TensorE recovers bucket j by linearity: output column
    (b8,j) accumulates +w*D_{j+1} - w*D_j (D_12 = p), so each (tile,j)
    costs one matmul pass and one DVE op, and the bucket difference is
    bit-exact (D planes share p's bf16 bits).  The +-w stationaries are
    Pool-built from host +-1 patterns after the collective.  Bucket
    sums [96=(b8*12+j), k] are copied to bf16 and DMA'd out; the host
    interleaves them into [B, V+1, 2] (channel 1 is a constant iota).

  All large inputs load with one DMA each (HWDGE is shared and serial,
  ~650ns per dma_start).
"""

import functools
import numpy as np

# ---- problem constants (hardcoded per contract) ----
V = 50257
E, B, K, D = 16, 256, 4097, 1024
HID = [512, 256, 128]
EPS = 1e-6
NCORES = 8
BL = B // NCORES          # 32 local batch rows per core
ST = 12                   # V // K  (index stride)
KU = K - 1                # 4096 used k slots
VU = KU * ST              # 49152 used vocab columns
NB8 = 8                   # batch rows per partition group
NBG = BL // NB8           # 4 batch groups
KT = 2048                 # k-tile
NKT = KU // KT            # 2
HK = 1024                 # half-tile k extent (PSUM half for double buffer)
PS = 512                  # psum free slice (one bank of fp32)
ROWS = E * BL             # 512 MLP rows
PCOL = NB8 * ST           # 96 = (b8, j) output columns of the e-sum matmul
NPASS = ST                # 12 moving passes per tile (D_1..D_11 + probs)
N_WARM0 = 8               # pre-MLP PE warmup matmuls (ramp to full p-state)
N_WARMG = 8               # gap-filler warmups at layer transitions
N_WARM1 = 80              # collective-gap PE warmup matmuls


def _build_program(use_bias=False):
    from concourse import bacc
    from concourse import bass
    from concourse import tile
    import concourse.mybir as mybir

    f32 = mybir.dt.float32
    bf16 = mybir.dt.bfloat16
    AF = mybir.ActivationFunctionType
    OP = mybir.AluOpType
    X = mybir.AxisListType.X

    nc = bacc.Bacc(
        "TRN2",
        target_bir_lowering=False,
        debug=False,
        enable_asserts=False,
        num_devices=NCORES,
    )

    # ---- kernel I/O (weights pre-chunked on host: one DMA per tensor) ----
    emb = nc.declare_dram_parameter("emb", [128, 8 * ROWS], bf16, isOutput=False)
    probs_p = nc.declare_dram_parameter("probs", [NBG, 128, NKT * KT], bf16, isOutput=False)
    offs_p = nc.declare_dram_parameter("offs", [NBG, 128, NKT * KT], bf16, isOutput=False)
    w1 = nc.declare_dram_parameter("w1", [128, 8 * HID[0]], bf16, isOutput=False)
    w2 = nc.declare_dram_parameter("w2", [128, 4 * HID[1]], bf16, isOutput=False)
    w3 = nc.declare_dram_parameter("w3", [128, 2 * HID[2]], bf16, isOutput=False)
    wo = nc.declare_dram_parameter("wo", [128, 1], bf16, isOutput=False)
    b1r = nc.declare_dram_parameter("b1r", [128, HID[0]], f32, isOutput=False)
    b2r = nc.declare_dram_parameter("b2r", [128, HID[1]], f32, isOutput=False)
    b3r = nc.declare_dram_parameter("b3r", [128, HID[2]], f32, isOutput=False)
    wpat = nc.declare_dram_parameter("wpat", [128, NPASS * PCOL], bf16, isOutput=False)
    identb = nc.declare_dram_parameter("identb", [128, 128], bf16, isOutput=False)
    out = nc.declare_dram_parameter("out", [NBG, NKT, PCOL, KT], bf16, isOutput=True)

    NH = [D] + HID  # 1024, 512, 256, 128

    with tile.TileContext(nc) as tc:
        with (
            tc.tile_pool(name="const", bufs=1) as cp,
            tc.tile_pool(name="dram", bufs=1, space="DRAM") as dp,
            tc.tile_pool(name="mlp", bufs=1) as mp,
            tc.tile_pool(name="mpsum", bufs=3, space="PSUM") as mpsum,
            tc.tile_pool(name="wpsum", bufs=1, space="PSUM") as wpsum,
            tc.tile_pool(name="sc", bufs=1) as scp,
            tc.tile_pool(name="espsum", bufs=2, space="PSUM") as espsum,
        ):
            # ================= constants =================
            idb = cp.tile([128, 128], bf16, tag="idb")
            nc.sync.dma_start(out=idb[:], in_=identb[:])
            zbias = cp.tile([128, 1], f32, tag="zbias")
            nc.vector.memset(zbias[:], 0.0)
            thr = cp.tile([128, NPASS], f32, tag="thr")
            for t in range(NPASS):
                nc.vector.memset(thr[:, t:t + 1], t - 0.5)

            # single-DMA loads
            embt = cp.tile([128, 8 * ROWS], bf16, tag="embt")
            nc.sync.dma_start(out=embt[:], in_=emb[:])
            w1t = cp.tile([128, 8 * HID[0]], bf16, tag="w1t")
            nc.sync.dma_start(out=w1t[:], in_=w1[:])
            w2t = cp.tile([128, 4 * HID[1]], bf16, tag="w2t")
            nc.sync.dma_start(out=w2t[:], in_=w2[:])
            w3t = cp.tile([128, 2 * HID[2]], bf16, tag="w3t")
            nc.sync.dma_start(out=w3t[:], in_=w3[:])
            wot = cp.tile([128, 1], bf16, tag="wot")
            nc.sync.dma_start(out=wot[:], in_=wo[:])
            wpatt = cp.tile([128, NPASS * PCOL], bf16, tag="wpatt")
            nc.sync.dma_start(out=wpatt[:], in_=wpat[:])

            h0T = [embt[:, c * ROWS:(c + 1) * ROWS] for c in range(8)]
            w1b = [w1t[:, c * HID[0]:(c + 1) * HID[0]] for c in range(8)]
            w2b = [w2t[:, c * HID[1]:(c + 1) * HID[1]] for c in range(4)]
            w3b = [w3t[:, c * HID[2]:(c + 1) * HID[2]] for c in range(2)]

            btiles = {1: b1r, 2: b2r, 3: b3r}
            brep = {}
            if use_bias:
                for li, dsz in ((1, HID[0]), (2, HID[1]), (3, HID[2])):
                    bt = cp.tile([128, dsz], f32, tag=f"brep{li}")
                    nc.sync.dma_start(out=bt[:], in_=btiles[li][:])
                    brep[li] = bt

            # pre-MLP PE warmup: ramp the p-state while emb loads
            junk = cp.tile([128, PS], bf16, tag="junk")
            nc.vector.memset(junk[:], 0.0)
            warm_ps = wpsum.tile([128, PS], f32, tag="warm")

            def warm(n):
                for _ in range(n):
                    nc.tensor.matmul(warm_ps[:], idb[:], junk[:],
                                     start=True, stop=True,
                                     skip_group_check=True)

            warm(N_WARM0)

            # ================= MLP =================
            def elu_ln(psum_z, li, dsz, rc):
                """psum [128, dsz] -> normalized bf16 tile [128, dsz].

                elu(z) = max(z, min(exp(z)-1, 0)); LN via E[h]/E[h^2] so
                the stats run concurrently; normalize is one dual-ptr
                tensor_scalar.  Act only runs Exp/Square/Copy (one table).
                """
                if use_bias:
                    zb = mp.tile([128, dsz], f32, tag="eln_zb", bufs=3)
                    nc.scalar.copy(zb[:], psum_z[:])
                    nc.vector.tensor_add(zb[:], zb[:], brep[li][:])
                    zsrc = zb
                else:
                    zsrc = psum_z
                e_t = mp.tile([128, dsz], f32, tag="eln_et", bufs=3)
                nc.scalar.activation(e_t[:], zsrc[:], AF.Exp, bias=zbias[:])
                m_t = mp.tile([128, dsz], f32, tag="eln_mt", bufs=3)
                nc.vector.tensor_scalar(m_t[:], e_t[:], 1.0, 0.0, OP.subtract, OP.min)
                h = mp.tile([128, dsz], f32, tag="eln_h", bufs=3)
                nc.vector.tensor_tensor(h[:], m_t[:], zsrc[:], OP.max)
                s = mp.tile([128, 1], f32, tag="eln_s")
                nc.vector.tensor_reduce(s[:], h[:], X, OP.add)
                sq = mp.tile([128, dsz], f32, tag="eln_sq", bufs=2)
                ss2 = mp.tile([128, 1], f32, tag="eln_ss2")
                nc.scalar.activation(sq[:], h[:], AF.Square, bias=zbias[:], accum_out=ss2[:])
                mu = mp.tile([128, 1], f32, tag="eln_mu")
                nc.vector.tensor_scalar(mu[:], s[:], 1.0 / dsz, None, OP.mult)
                # var = (ss2 - s^2/dsz) / (dsz-1)
                v1 = mp.tile([128, 1], f32, tag="eln_v1")
                nc.vector.tensor_scalar(v1[:], s[:], s[:], 1.0 / dsz, OP.mult, OP.mult)
                v2 = mp.tile([128, 1], f32, tag="eln_v2")
                nc.vector.tensor_tensor(v2[:], ss2[:], v1[:], OP.subtract)
                sd = mp.tile([128, 1], f32, tag="eln_sd")
                nc.vector.tensor_scalar(sd[:], v2[:], 1.0 / (dsz - 1), 0.5, OP.mult, OP.pow)
                rcp = mp.tile([128, 1], f32, tag="eln_rcp")
                nc.vector.reciprocal(rcp[:], sd[:])
                hn = mp.tile([128, dsz], bf16, tag=f"hn{li}_{rc}")
                nc.vector.tensor_scalar(hn[:], h[:], mu[:], rcp[:], OP.subtract, OP.mult)
                return hn

            def layer_mm(hT_of_rc, wtiles, rc, d_in, d_out):
                """One rc-chunk's matmuls: stationary slices from hT_of_rc."""
                pz = mpsum.tile([128, d_out], f32, tag="mt")
                nk = d_in // 128
                for fc in range(nk):
                    nc.tensor.matmul(
                        pz[:],
                        hT_of_rc[fc],
                        wtiles[fc][:],
                        start=(fc == 0),
                        stop=(fc == nk - 1),
                    )
                return pz

            def trans_rc(h_rc, d_feat, name, rc):
                """h_rc [128 rows, d_feat] -> per-fc [128 feat, 128 rows] slices."""
                t = mp.tile([128, (d_feat // 128) * 128], bf16, tag=f"{name}T{rc}")
                outs = []
                for fc in range(d_feat // 128):
                    pt = mpsum.tile([128, 128], bf16, tag="mt")
                    nc.tensor.transpose(
                        pt[:], h_rc[:, fc * 128:(fc + 1) * 128], idb[:])
                    sl = t[:, fc * 128:(fc + 1) * 128]
                    nc.scalar.copy(sl, pt[:])
                    outs.append(sl)
                return outs

            # layer 1 (stationary = pre-transposed emb chunks)
            h1 = []
            for rc in range(4):
                hT_rc = [h0T[fc][:, rc * 128:(rc + 1) * 128] for fc in range(8)]
                pz = layer_mm(hT_rc, w1b, rc, NH[0], NH[1])
                h1.append(elu_ln(pz, 1, NH[1], rc))
            # layers 2/3 pipelined per rc-chunk through transpose; a few
            # gap-filler warmups keep the p-state ramp alive across the
            # eln-latency bubbles at layer transitions
            h2 = []
            for rc in range(4):
                hT_rc = trans_rc(h1[rc][:], NH[1], "h1", rc)
                pz = layer_mm(hT_rc, w2b, rc, NH[1], NH[2])
                h2.append(elu_ln(pz, 2, NH[2], rc))
            warm(N_WARMG)
            h3 = []
            for rc in range(4):
                hT_rc = trans_rc(h2[rc][:], NH[2], "h2", rc)
                pz = layer_mm(hT_rc, w3b, rc, NH[2], NH[3])
                h3.append(elu_ln(pz, 3, NH[3], rc))
            warm(N_WARMG)
            # scores need h3T[fc=0] spanning all rc
            h3T = mp.tile([128, ROWS], bf16, tag="h3T")
            for rc in range(4):
                pt = mpsum.tile([128, 128], bf16, tag="mt")
                nc.tensor.transpose(pt[:], h3[rc][:, :128], idb[:])
                nc.scalar.copy(h3T[:, rc * 128:(rc + 1) * 128], pt[:])

            ps_s = mpsum.tile([1, ROWS], f32, tag="mt")
            nc.tensor.matmul(ps_s[:], wot[:], h3T[:], start=True, stop=True)
            scores = mp.tile([1, ROWS], f32, tag="scores")
            nc.scalar.copy(scores[:], ps_s[:])

            # gate the threshold planes on MLP completion: a zero tile that
            # data-depends on h3 keeps DVE free for the eln chains until the
            # MLP is done (the scheduler would otherwise starve them behind
            # ready mask ops)
            z128 = mp.tile([128, 1], f32, tag="z128")
            nc.scalar.activation(z128[:], h3[3][:, :1], AF.Copy, bias=0.0,
                                 scale=0.0)
            thrf = cp.tile([128, NPASS], f32, tag="thrf")
            nc.vector.tensor_scalar(thrf[:], thr[:], z128[:], None, OP.add)

            # ---- softmax over full batch, DVE-free (DVE is busy with
            # threshold planes): scores -> [E, BL] partitions, Act exp
            # with accum gives both w16 and the per-expert partials ----
            s2 = mp.tile([E, BL], f32, tag="s2")
            nc.sync.dma_start(out=s2[:], in_=scores[:1, :])
            w16 = mp.tile([E, BL], f32, tag="w16")
            smy16 = mp.tile([E, 1], f32, tag="smy16")
            nc.scalar.activation(w16[:], s2[:], AF.Exp, bias=zbias[:E, :],
                                 accum_out=smy16[:])
            cc_in = dp.tile([E, 1], f32, tag="ccin")
            cc_out = dp.tile([NCORES, E], f32, tag="ccout")
            nc.sync.dma_start(out=cc_in[:], in_=smy16[:])
            nc.gpsimd.collective_compute(
                "AllGather",
                OP.bypass,
                replica_groups=[list(range(NCORES))],
                ins=[cc_in[:].opt()],
                outs=[cc_out[:].opt()],
            )
            # gathered partials, transposed to [E, NCORES]; Act accum sums
            sgT = mp.tile([E, NCORES], f32, tag="sgT")
            nc.sync.dma_start(out=sgT[:], in_=cc_out[:].rearrange("c e -> e c"))
            sgc = mp.tile([E, NCORES], f32, tag="sgc")
            s16 = mp.tile([E, 1], f32, tag="s16")
            nc.scalar.activation(sgc[:], sgT[:], AF.Copy, bias=0.0,
                                 accum_out=s16[:])
            rcp16 = mp.tile([E, 1], f32, tag="rcp16")
            nc.gpsimd.tensor_scalar(rcp16[:], s16[:], -1.0, None, OP.pow)
            wmy = mp.tile([E, BL], f32, tag="wmy")
            nc.gpsimd.tensor_scalar(wmy[:], w16[:], rcp16[:], None, OP.mult)
            w_pp = cp.tile([128, NBG], f32, tag="wpp")
            for bg in range(NBG):
                nc.sync.dma_start(
                    out=w_pp[:, bg:bg + 1],
                    in_=wmy[:, bg * NB8:(bg + 1) * NB8],
                )
            # 48 routing-weighted stationaries [128, PCOL], bg-major so the
            # first tile's passes unblock as early as possible
            stat = [[None] * (NPASS + 1) for _ in range(NBG)]
            for bg in range(NBG):
                for t in range(1, NPASS + 1):
                    st_t = cp.tile([128, PCOL], bf16, tag=f"stat{bg}_{t}")
                    nc.gpsimd.tensor_scalar(
                        st_t[:], wpatt[:, (t - 1) * PCOL:t * PCOL],
                        w_pp[:, bg:bg + 1], None, OP.mult)
                    stat[bg][t] = st_t

            # keep the PE p-state hot through the collective gap
            warm(N_WARM1)

            # ================= scatter =================
            # D_t = p * 1[offs < t] for t=1..11; pass 12 = raw probs.
            # Column (b8,j) of es accumulates +w*D_{j+1} - w*D_j.
            for bg in range(NBG):
                prb = scp.tile([128, NKT * KT], bf16, tag="prb", bufs=2)
                nc.sync.dma_start(out=prb[:], in_=probs_p[bg])
                ofs = scp.tile([128, NKT * KT], bf16, tag="ofs", bufs=2)
                nc.sync.dma_start(out=ofs[:], in_=offs_p[bg])
                for kt in range(NKT):
                    ksl = slice(kt * KT, (kt + 1) * KT)
                    planes = [None] * (NPASS + 1)
                    for t in range(1, NPASS - 1):
                        d_t = scp.tile([128, KT], bf16, tag=f"D{t}", bufs=2)
                        nc.vector.tensor_mask(
                            d_t[:], prb[:, ksl], thrf[:, t:t + 1], ofs[:, ksl], 0)
                        planes[t] = d_t[:]
                    # D_11 on the (post-collective idle) Pool engine
                    t = NPASS - 1
                    mskp = scp.tile([128, KT], bf16, tag="mskp", bufs=2)
                    nc.gpsimd.tensor_scalar(
                        mskp[:], ofs[:, ksl], t - 0.5, None, OP.is_lt)
                    d11 = scp.tile([128, KT], bf16, tag=f"D{t}", bufs=2)
                    nc.gpsimd.tensor_tensor(
                        d11[:], mskp[:], prb[:, ksl], OP.mult)
                    planes[t] = d11[:]
                    planes[NPASS] = prb[:, ksl]
                    ob = scp.tile([PCOL, KT], bf16, tag="ob", bufs=2)
                    for h in range(2):
                        es = espsum.tile([PCOL, HK], f32, tag="es")
                        for t in range(1, NPASS + 1):
                            for s in range(HK // PS):
                                lo = h * HK + s * PS
                                nc.tensor.matmul(
                                    es[:, s * PS:(s + 1) * PS],
                                    stat[bg][t][:],
                                    planes[t][:, lo:lo + PS],
                                    start=(t == 1),
                                    stop=(t == NPASS),
                                    skip_group_check=True,
                                )
                        nc.scalar.copy(ob[:, h * HK:(h + 1) * HK], es[:])
                    nc.sync.dma_start(out=out[bg, kt], in_=ob[:])
    nc.compile()
    return nc


@functools.lru_cache(maxsize=2)
def _program(use_bias=False):
    return _build_program(use_bias)


def _chunk(a, nch):
    """[nch*128, dout] f32 -> [128, nch*dout] bf16 (chunk-major free)."""
    import ml_dtypes
    dout = a.shape[1]
    return np.ascontiguousarray(
        a.reshape(nch, 128, dout).transpose(1, 0, 2).reshape(128, nch * dout)
    ).astype(ml_dtypes.bfloat16)


def _host_prep(inputs):
    """Fold LN affine params into following layers; build constants."""
    import ml_dtypes
    f32 = np.float32
    bf = ml_dtypes.bfloat16
    W1 = inputs["W1"].astype(np.float64)
    W2 = inputs["W2"].astype(np.float64)
    W3 = inputs["W3"].astype(np.float64)
    Wout = inputs["Wout"].astype(np.float64)
    g1, be1 = inputs["g1"].astype(np.float64), inputs["be1"].astype(np.float64)
    g2, be2 = inputs["g2"].astype(np.float64), inputs["be2"].astype(np.float64)
    g3 = inputs["g3"].astype(np.float64)
    b1, b2, b3 = (inputs["b1"].astype(np.float64), inputs["b2"].astype(np.float64),
                  inputs["b3"].astype(np.float64))

    w1f = W1
    b1f = b1
    w2f = g1[:, None] * W2
    b2f = b2 + be1 @ W2
    w3f = g2[:, None] * W3
    b3f = b3 + be2 @ W3
    wof = g3[:, None] * Wout
    # bout / be3@Wout shift all scores equally -> softmax-invariant, dropped.

    consts = {
        "w1": _chunk(w1f.astype(f32), 8),
        "w2": _chunk(w2f.astype(f32), 4),
        "w3": _chunk(w3f.astype(f32), 2),
        "wo": wof.astype(f32).astype(bf),
        "b1r": np.broadcast_to(b1f.astype(f32), (128, HID[0])).copy(),
        "b2r": np.broadcast_to(b2f.astype(f32), (128, HID[1])).copy(),
        "b3r": np.broadcast_to(b3f.astype(f32), (128, HID[2])).copy(),
    }

    # +-1 patterns: pass t feeds column (b8, t-1) with +1 and column
    # (b8, t) with -1 (pass 12 = raw probs only feeds column 11).
    wpat = np.zeros((NPASS, 128, PCOL), f32)
    for t in range(1, NPASS + 1):
        for e in range(E):
            for b8 in range(NB8):
                p = e * NB8 + b8
                wpat[t - 1, p, b8 * ST + (t - 1)] = 1.0
                if t < NPASS:
                    wpat[t - 1, p, b8 * ST + t] = -1.0
    consts["wpat"] = np.ascontiguousarray(
        wpat.transpose(1, 0, 2).reshape(128, NPASS * PCOL)).astype(bf)
    consts["identb"] = np.eye(128, dtype=f32).astype(bf)
    return consts


LAST_RESULTS = None


def _core_inputs(consts, emb_full, pred_full, c):
    import ml_dtypes
    bf = ml_dtypes.bfloat16
    bsl = slice(c * BL, (c + 1) * BL)
    m = dict(consts)
    embT = np.ascontiguousarray(
        emb_full[:, bsl, :].reshape(ROWS, D).T)          # [D, ROWS] f32
    m["emb"] = _chunk(embT, 8)
    pc = pred_full[:, bsl, :KU, :]                       # [E, 32, KU, 2]
    probs = pc[..., 0].astype(bf)
    offs_i = (pc[..., 1].astype(np.int32)
              - ST * np.arange(KU, dtype=np.int32)[None, None, :])
    # structural contract of the generator: idx = 12*k + offs, offs in [0,12)
    assert offs_i.min() >= 0 and offs_i.max() < ST, (
        "index structure violated: idx != 12*k + offs")
    offs = offs_i.astype(bf)
    def shuf(a):
        a = a.reshape(E, NBG, NB8, KU)
        return np.ascontiguousarray(
            a.transpose(1, 0, 2, 3).reshape(NBG, 128, KU))
    m["probs"] = shuf(probs)
    m["offs"] = shuf(offs)
    return m


def kernel(**inputs) -> np.ndarray:
    from concourse.bass_utils import run_bass_kernel_spmd

    inputs = {k: np.asarray(v) for k, v in inputs.items()}
    consts = _host_prep(inputs)
    use_bias = any(
        np.abs(consts[k]).max() > 0 for k in ("b1r", "b2r", "b3r"))
    nc = _program(use_bias)

    emb_full = np.asarray(inputs["endpoint_emb"], np.float32)
    pred_full = np.asarray(inputs["prediction"], np.float32)

    in_maps = [_core_inputs(consts, emb_full, pred_full, c)
               for c in range(NCORES)]

    res = run_bass_kernel_spmd(nc, in_maps, core_ids=list(range(NCORES)))
    global LAST_RESULTS
    LAST_RESULTS = res

    outf = np.zeros((B, V + 1, 2), np.float32)
    outf[:, :V, 1] = np.arange(V, dtype=np.float32)
    outf[:, V, 1] = -1.0
    for c in range(NCORES):
        # device out: [bg, kt, (b8*12+j), kk] -> [b, (kt,kk,j)]
        o = np.asarray(res.results[c]["out"], np.float32)
        o = o.reshape(NBG, NKT, NB8, ST, KT)
        o = o.transpose(0, 2, 1, 4, 3).reshape(BL, VU)
        outf[c * BL:(c + 1) * BL, :VU, 0] = o
    return outf


# revision 29
# speedup vs baseline: 1.0105x; 1.0105x over previous
"""Trainium2 Bass kernel for nn_BaseMOE (moe_routing), 8 NeuronCores.

Batch-sharded (B=256 -> 32 rows/core); full inputs in, full output out.

Per core:
  * 3-layer MLP + Wout on its [16 experts x 32 batch] rows in bf16.
    LayerNorm affine folded into the next layer's weights on the host;
    ELU via h = max(z, min(exp(z)-1, 0)); PSUM released early through an
    Activation-engine copy; LN sqrt/reciprocal on DVE so the Activation
    engine never swaps function tables; dummy matmuls warm the PE
    p-state before the MLP and through the collective gap.
  * softmax-over-batch: local exp(scores), per-expert partial sums
    exchanged with a 64-byte AllGather; all post-collective weight prep
    runs on Pool so the DVE queue (busy with threshold planes) never
    blocks on it.
  * scatter: idx[e,b,k] = 12*k + offs, offs in [0,12).  DVE builds 11
    *threshold* planes D_t = p * 1[offs < t] per [128,2048] tile with
    single tensor_mask ops (2x mode); the raw probs tile is the 12th
    plane.  TensorE recovers bucket j by linearity: output column
    (b8,j) accumulates +w*D_{j+1} - w*D_j (D_12 = p), so each (tile,j)
    costs one matmul pass and one DVE op, and the bucket difference is
    bit-exact (D planes share p's bf16 bits).  The +-w stationaries are
    Pool-built from host +-1 patterns after the collective.  Bucket
    sums [96=(b8*12+j), k] are copied to bf16 and DMA'd out; the host
    interleaves them into [B, V+1, 2] (channel 1 is a constant iota).

  All large inputs load with one DMA each (HWDGE is shared and serial,
  ~650ns per dma_start).
"""

import functools
import numpy as np

# ---- problem constants (hardcoded per contract) ----
V = 50257
E, B, K, D = 16, 256, 4097, 1024
HID = [512, 256, 128]
EPS = 1e-6
NCORES = 8
BL = B // NCORES          # 32 local batch rows per core
ST = 12                   # V // K  (index stride)
KU = K - 1                # 4096 used k slots
VU = KU * ST              # 49152 used vocab columns
NB8 = 8                   # batch rows per partition group
NBG = BL // NB8           # 4 batch groups
KT = 2048                 # k-tile
NKT = KU // KT            # 2
HK = 1024                 # half-tile k extent (PSUM half for double buffer)
PS = 512                  # psum free slice (one bank of fp32)
ROWS = E * BL             # 512 MLP rows
PCOL = NB8 * ST           # 96 = (b8, j) output columns of the e-sum matmul
NPASS = ST                # 12 moving passes per tile (D_1..D_11 + probs)
N_WARM1 = 85              # collective-gap PE warmup matmuls (keep p-state hot)


def _build_program(use_bias=False):
    from concourse import bacc
    from concourse import bass
    from concourse import tile
    import concourse.mybir as mybir

    f32 = mybir.dt.float32
    bf16 = mybir.dt.bfloat16
    AF = mybir.ActivationFunctionType
    OP = mybir.AluOpType
    X = mybir.AxisListType.X

    nc = bacc.Bacc(
        "TRN2",
        target_bir_lowering=False,
        debug=False,
        enable_asserts=False,
        num_devices=NCORES,
    )

    # ---- kernel I/O (weights pre-chunked on host: one DMA per tensor) ----
    emb = nc.declare_dram_parameter("emb", [128, 8 * ROWS], bf16, isOutput=False)
    probs_p = nc.declare_dram_parameter("probs", [NBG, 128, NKT * KT], bf16, isOutput=False)
    offs_p = nc.declare_dram_parameter("offs", [NBG, 128, NKT * KT], bf16, isOutput=False)
    w1 = nc.declare_dram_parameter("w1", [128, 8 * HID[0]], bf16, isOutput=False)
    w2 = nc.declare_dram_parameter("w2", [128, 4 * HID[1]], bf16, isOutput=False)
    w3 = nc.declare_dram_parameter("w3", [128, 2 * HID[2]], bf16, isOutput=False)
    wo = nc.declare_dram_parameter("wo", [128, 1], bf16, isOutput=False)
    b1r = nc.declare_dram_parameter("b1r", [128, HID[0]], f32, isOutput=False)
    b2r = nc.declare_dram_parameter("b2r", [128, HID[1]], f32, isOutput=False)
    b3r = nc.declare_dram_parameter("b3r", [128, HID[2]], f32, isOutput=False)
    wpat = nc.declare_dram_parameter("wpat", [128, NPASS * PCOL], bf16, isOutput=False)
    identb = nc.declare_dram_parameter("identb", [128, 128], bf16, isOutput=False)
    out = nc.declare_dram_parameter("out", [NBG, NKT, PCOL, KT], bf16, isOutput=True)

    NH = [D] + HID  # 1024, 512, 256, 128

    with tile.TileContext(nc) as tc:
        with (
            tc.tile_pool(name="const", bufs=1) as cp,
            tc.tile_pool(name="dram", bufs=1, space="DRAM") as dp,
            tc.tile_pool(name="mlp", bufs=1) as mp,
            tc.tile_pool(name="mpsum", bufs=3, space="PSUM") as mpsum,
            tc.tile_pool(name="wpsum", bufs=1, space="PSUM") as wpsum,
            tc.tile_pool(name="sc", bufs=1) as scp,
            tc.tile_pool(name="espsum", bufs=2, space="PSUM") as espsum,
        ):
            # ================= constants =================
            idb = cp.tile([128, 128], bf16, tag="idb")
            nc.sync.dma_start(out=idb[:], in_=identb[:])
            zbias = cp.tile([128, 1], f32, tag="zbias")
            nc.vector.memset(zbias[:], 0.0)
            thr = cp.tile([128, NPASS], f32, tag="thr")
            for t in range(NPASS):
                nc.vector.memset(thr[:, t:t + 1], t - 0.5)

            # few-DMA loads (emb/w1 halved so layer 1 starts sooner)
            embt = cp.tile([128, 8 * ROWS], bf16, tag="embt")
            w1t = cp.tile([128, 8 * HID[0]], bf16, tag="w1t")
            HB = 4 * ROWS
            HW1 = 4 * HID[0]
            nc.sync.dma_start(out=embt[:, :HB], in_=emb[:, :HB])
            nc.sync.dma_start(out=w1t[:, :HW1], in_=w1[:, :HW1])
            nc.sync.dma_start(out=embt[:, HB:], in_=emb[:, HB:])
            nc.sync.dma_start(out=w1t[:, HW1:], in_=w1[:, HW1:])
            w2t = cp.tile([128, 4 * HID[1]], bf16, tag="w2t")
            nc.sync.dma_start(out=w2t[:], in_=w2[:])
            w3t = cp.tile([128, 2 * HID[2]], bf16, tag="w3t")
            nc.sync.dma_start(out=w3t[:], in_=w3[:])
            wot = cp.tile([128, 1], bf16, tag="wot")
            nc.sync.dma_start(out=wot[:], in_=wo[:])
            wpatt = cp.tile([128, NPASS * PCOL], bf16, tag="wpatt")
            nc.sync.dma_start(out=wpatt[:], in_=wpat[:])

            h0T = [embt[:, c * ROWS:(c + 1) * ROWS] for c in range(8)]
            w1b = [w1t[:, c * HID[0]:(c + 1) * HID[0]] for c in range(8)]
            w2b = [w2t[:, c * HID[1]:(c + 1) * HID[1]] for c in range(4)]
            w3b = [w3t[:, c * HID[2]:(c + 1) * HID[2]] for c in range(2)]

            btiles = {1: b1r, 2: b2r, 3: b3r}
            brep = {}
            if use_bias:
                for li, dsz in ((1, HID[0]), (2, HID[1]), (3, HID[2])):
                    bt = cp.tile([128, dsz], f32, tag=f"brep{li}")
                    nc.sync.dma_start(out=bt[:], in_=btiles[li][:])
                    brep[li] = bt

            junk = cp.tile([128, PS], bf16, tag="junk")
            nc.vector.memset(junk[:], 0.0)
            warm_ps = wpsum.tile([128, PS], f32, tag="warm")

            # ================= MLP =================
            def elu_ln(psum_z, li, dsz, rc):
                """psum [128, dsz] -> normalized bf16 tile [128, dsz].

                elu(z) = max(z, min(exp(z)-1, 0)); LN via E[h]/E[h^2] so
                the stats run concurrently; normalize is one dual-ptr
                tensor_scalar.  Act only runs Exp/Square/Copy (one table).
                """
                if use_bias:
                    zb = mp.tile([128, dsz], f32, tag="eln_zb", bufs=3)
                    nc.scalar.copy(zb[:], psum_z[:])
                    nc.vector.tensor_add(zb[:], zb[:], brep[li][:])
                    zsrc = zb
                else:
                    zsrc = psum_z
                e_t = mp.tile([128, dsz], f32, tag="eln_et", bufs=3)
                nc.scalar.activation(e_t[:], zsrc[:], AF.Exp, bias=zbias[:])
                m_t = mp.tile([128, dsz], f32, tag="eln_mt", bufs=3)
                nc.vector.tensor_scalar(m_t[:], e_t[:], 1.0, 0.0, OP.subtract, OP.min)
                h = mp.tile([128, dsz], f32, tag="eln_h", bufs=3)
                nc.vector.tensor_tensor(h[:], m_t[:], zsrc[:], OP.max)
                s = mp.tile([128, 1], f32, tag="eln_s")
                nc.vector.tensor_reduce(s[:], h[:], X, OP.add)
                sq = mp.tile([128, dsz], f32, tag="eln_sq", bufs=2)
                ss2 = mp.tile([128, 1], f32, tag="eln_ss2")
                nc.scalar.activation(sq[:], h[:], AF.Square, bias=zbias[:], accum_out=ss2[:])
                mu = mp.tile([128, 1], f32, tag="eln_mu")
                nc.vector.tensor_scalar(mu[:], s[:], 1.0 / dsz, None, OP.mult)
                # var = (ss2 - s^2/dsz) / (dsz-1)
                v1 = mp.tile([128, 1], f32, tag="eln_v1")
                nc.vector.tensor_scalar(v1[:], s[:], s[:], 1.0 / dsz, OP.mult, OP.mult)
                v2 = mp.tile([128, 1], f32, tag="eln_v2")
                nc.vector.tensor_tensor(v2[:], ss2[:], v1[:], OP.subtract)
                sd = mp.tile([128, 1], f32, tag="eln_sd")
                nc.vector.tensor_scalar(sd[:], v2[:], 1.0 / (dsz - 1), 0.5, OP.mult, OP.pow)
                rcp = mp.tile([128, 1], f32, tag="eln_rcp")
                nc.vector.reciprocal(rcp[:], sd[:])
                hn = mp.tile([128, dsz], bf16, tag=f"hn{li}_{rc}")
                nc.vector.tensor_scalar(hn[:], h[:], mu[:], rcp[:], OP.subtract, OP.mult)
                return hn

            def layer_mm(hT_of_rc, wtiles, rc, d_in, d_out):
                """One rc-chunk's matmuls: stationary slices from hT_of_rc."""
                pz = mpsum.tile([128, d_out], f32, tag="mt")
                nk = d_in // 128
                for fc in range(nk):
                    nc.tensor.matmul(
                        pz[:],
                        hT_of_rc[fc],
                        wtiles[fc][:],
                        start=(fc == 0),
                        stop=(fc == nk - 1),
                    )
                return pz

            def trans_rc(h_rc, d_feat, name, rc):
                """h_rc [128 rows, d_feat] -> per-fc [128 feat, 128 rows] slices."""
                t = mp.tile([128, (d_feat // 128) * 128], bf16, tag=f"{name}T{rc}")
                outs = []
                for fc in range(d_feat // 128):
                    pt = mpsum.tile([128, 128], bf16, tag="mt")
                    nc.tensor.transpose(
                        pt[:], h_rc[:, fc * 128:(fc + 1) * 128], idb[:])
                    sl = t[:, fc * 128:(fc + 1) * 128]
                    nc.scalar.copy(sl, pt[:])
                    outs.append(sl)
                return outs

            # layer 1 (stationary = pre-transposed emb chunks)
            h1 = []
            for rc in range(4):
                hT_rc = [h0T[fc][:, rc * 128:(rc + 1) * 128] for fc in range(8)]
                pz = layer_mm(hT_rc, w1b, rc, NH[0], NH[1])
                h1.append(elu_ln(pz, 1, NH[1], rc))
            # layers 2/3 pipelined per rc-chunk through transpose; a few
            # gap-filler warmups keep the p-state ramp alive across the
            # eln-latency bubbles at layer transitions
            h2 = []
            for rc in range(4):
                hT_rc = trans_rc(h1[rc][:], NH[1], "h1", rc)
                pz = layer_mm(hT_rc, w2b, rc, NH[1], NH[2])
                h2.append(elu_ln(pz, 2, NH[2], rc))
            h3 = []
            for rc in range(4):
                hT_rc = trans_rc(h2[rc][:], NH[2], "h2", rc)
                pz = layer_mm(hT_rc, w3b, rc, NH[2], NH[3])
                h3.append(elu_ln(pz, 3, NH[3], rc))
            # scores need h3T[fc=0] spanning all rc
            h3T = mp.tile([128, ROWS], bf16, tag="h3T")
            for rc in range(4):
                pt = mpsum.tile([128, 128], bf16, tag="mt")
                nc.tensor.transpose(pt[:], h3[rc][:, :128], idb[:])
                nc.scalar.copy(h3T[:, rc * 128:(rc + 1) * 128], pt[:])

            ps_s = mpsum.tile([1, ROWS], f32, tag="mt")
            nc.tensor.matmul(ps_s[:], wot[:], h3T[:], start=True, stop=True)
            scores = mp.tile([1, ROWS], f32, tag="scores")
            nc.scalar.copy(scores[:], ps_s[:])

            # gate the threshold planes on MLP completion: a zero tile that
            # data-depends on h3 keeps DVE free for the eln chains until the
            # MLP is done (the scheduler would otherwise starve them behind
            # ready mask ops)
            z128 = mp.tile([128, 1], f32, tag="z128")
            nc.scalar.activation(z128[:], h3[3][:, :1], AF.Copy, bias=0.0,
                                 scale=0.0)
            thrf = cp.tile([128, NPASS], f32, tag="thrf")
            nc.vector.tensor_scalar(thrf[:], thr[:], z128[:], None, OP.add)
            # warmups gated on MLP completion so they span the collective
            # window instead of padding the MLP's PE stream
            junk2 = cp.tile([128, PS], bf16, tag="junk2")
            nc.scalar.activation(junk2[:], junk[:], AF.Relu, bias=z128[:])

            # ---- softmax over full batch, DVE-free (DVE is busy with
            # threshold planes): scores -> [E, BL] partitions, Act exp
            # with accum gives both w16 and the per-expert partials ----
            s2 = mp.tile([E, BL], f32, tag="s2")
            nc.sync.dma_start(out=s2[:], in_=scores[:1, :])
            w16 = mp.tile([E, BL], f32, tag="w16")
            smy16 = mp.tile([E, 1], f32, tag="smy16")
            nc.scalar.activation(w16[:], s2[:], AF.Exp, bias=zbias[:E, :],
                                 accum_out=smy16[:])
            cc_in = dp.tile([E, 1], f32, tag="ccin")
            cc_out = dp.tile([NCORES, E], f32, tag="ccout")
            nc.sync.dma_start(out=cc_in[:], in_=smy16[:])
            nc.gpsimd.collective_compute(
                "AllGather",
                OP.bypass,
                replica_groups=[list(range(NCORES))],
                ins=[cc_in[:].opt()],
                outs=[cc_out[:].opt()],
            )
            # gathered partials, transposed to [E, NCORES]; Act accum sums
            sgT = mp.tile([E, NCORES], f32, tag="sgT")
            nc.sync.dma_start(out=sgT[:], in_=cc_out[:].rearrange("c e -> e c"))
            sgc = mp.tile([E, NCORES], f32, tag="sgc")
            s16 = mp.tile([E, 1], f32, tag="s16")
            nc.scalar.activation(sgc[:], sgT[:], AF.Copy, bias=0.0,
                                 accum_out=s16[:])
            rcp16 = mp.tile([E, 1], f32, tag="rcp16")
            nc.gpsimd.tensor_scalar(rcp16[:], s16[:], -1.0, None, OP.pow)
            wmy = mp.tile([E, BL], f32, tag="wmy")
            nc.gpsimd.tensor_scalar(wmy[:], w16[:], rcp16[:], None, OP.mult)
            w_pp = cp.tile([128, NBG], f32, tag="wpp")
            for bg in range(NBG):
                nc.sync.dma_start(
                    out=w_pp[:, bg:bg + 1],
                    in_=wmy[:, bg * NB8:(bg + 1) * NB8],
                )
            # 48 routing-weighted stationaries [128, PCOL], bg-major so the
            # first tile's passes unblock as early as possible
            stat = [[None] * (NPASS + 1) for _ in range(NBG)]
            for bg in range(NBG):
                for t in range(1, NPASS + 1):
                    st_t = cp.tile([128, PCOL], bf16, tag=f"stat{bg}_{t}")
                    nc.gpsimd.tensor_scalar(
                        st_t[:], wpatt[:, (t - 1) * PCOL:t * PCOL],
                        w_pp[:, bg:bg + 1], None, OP.mult)
                    stat[bg][t] = st_t

            # keep the PE p-state hot through the collective gap
            for _ in range(N_WARM1):
                nc.tensor.matmul(warm_ps[:], idb[:], junk2[:],
                                 start=True, stop=True, skip_group_check=True)

            # ================= scatter =================
            # D_t = p * 1[offs < t] for t=1..11; pass 12 = raw probs.
            # Column (b8,j) of es accumulates +w*D_{j+1} - w*D_j.
            for bg in range(NBG):
                prb = scp.tile([128, NKT * KT], bf16, tag="prb", bufs=2)
                nc.sync.dma_start(out=prb[:], in_=probs_p[bg])
                ofs = scp.tile([128, NKT * KT], bf16, tag="ofs", bufs=2)
                nc.sync.dma_start(out=ofs[:], in_=offs_p[bg])
                for kt in range(NKT):
                    ksl = slice(kt * KT, (kt + 1) * KT)
                    planes = [None] * (NPASS + 1)
                    for t in range(1, NPASS - 1):
                        d_t = scp.tile([128, KT], bf16, tag=f"D{t}", bufs=2)
                        nc.vector.tensor_mask(
                            d_t[:], prb[:, ksl], thrf[:, t:t + 1], ofs[:, ksl], 0)
                        planes[t] = d_t[:]
                    # D_11 on the (post-collective idle) Pool engine
                    t = NPASS - 1
                    mskp = scp.tile([128, KT], bf16, tag="mskp", bufs=2)
                    nc.gpsimd.tensor_scalar(
                        mskp[:], ofs[:, ksl], t - 0.5, None, OP.is_lt)
                    d11 = scp.tile([128, KT], bf16, tag=f"D{t}", bufs=2)
                    nc.gpsimd.tensor_tensor(
                        d11[:], mskp[:], prb[:, ksl], OP.mult)
                    planes[t] = d11[:]
                    planes[NPASS] = prb[:, ksl]
                    ob = scp.tile([PCOL, KT], bf16, tag="ob", bufs=2)
                    for h in range(2):
                        es = espsum.tile([PCOL, HK], f32, tag="es")
                        for t in range(1, NPASS + 1):
                            for s in range(HK // PS):
                                lo = h * HK + s * PS
                                nc.tensor.matmul(
                                    es[:, s * PS:(s + 1) * PS],
                                    stat[bg][t][:],
                                    planes[t][:, lo:lo + PS],
                                    start=(t == 1),
                                    stop=(t == NPASS),
                                    skip_group_check=True,
                                )
                        nc.scalar.copy(ob[:, h * HK:(h + 1) * HK], es[:])
                    nc.sync.dma_start(out=out[bg, kt], in_=ob[:])
    nc.compile()
    return nc


@functools.lru_cache(maxsize=2)
def _program(use_bias=False):
    return _build_program(use_bias)


def _chunk(a, nch):
    """[nch*128, dout] f32 -> [128, nch*dout] bf16 (chunk-major free)."""
    import ml_dtypes
    dout = a.shape[1]
    return np.ascontiguousarray(
        a.reshape(nch, 128, dout).transpose(1, 0, 2).reshape(128, nch * dout)
    ).astype(ml_dtypes.bfloat16)


def _host_prep(inputs):
    """Fold LN affine params into following layers; build constants."""
    import ml_dtypes
    f32 = np.float32
    bf = ml_dtypes.bfloat16
    W1 = inputs["W1"].astype(np.float64)
    W2 = inputs["W2"].astype(np.float64)
    W3 = inputs["W3"].astype(np.float64)
    Wout = inputs["Wout"].astype(np.float64)
    g1, be1 = inputs["g1"].astype(np.float64), inputs["be1"].astype(np.float64)
    g2, be2 = inputs["g2"].astype(np.float64), inputs["be2"].astype(np.float64)
    g3 = inputs["g3"].astype(np.float64)
    b1, b2, b3 = (inputs["b1"].astype(np.float64), inputs["b2"].astype(np.float64),
                  inputs["b3"].astype(np.float64))

    w1f = W1
    b1f = b1
    w2f = g1[:, None] * W2
    b2f = b2 + be1 @ W2
    w3f = g2[:, None] * W3
    b3f = b3 + be2 @ W3
    wof = g3[:, None] * Wout
    # bout / be3@Wout shift all scores equally -> softmax-invariant, dropped.

    consts = {
        "w1": _chunk(w1f.astype(f32), 8),
        "w2": _chunk(w2f.astype(f32), 4),
        "w3": _chunk(w3f.astype(f32), 2),
        "wo": wof.astype(f32).astype(bf),
        "b1r": np.broadcast_to(b1f.astype(f32), (128, HID[0])).copy(),
        "b2r": np.broadcast_to(b2f.astype(f32), (128, HID[1])).copy(),
        "b3r": np.broadcast_to(b3f.astype(f32), (128, HID[2])).copy(),
    }

    # +-1 patterns: pass t feeds column (b8, t-1) with +1 and column
    # (b8, t) with -1 (pass 12 = raw probs only feeds column 11).
    wpat = np.zeros((NPASS, 128, PCOL), f32)
    for t in range(1, NPASS + 1):
        for e in range(E):
            for b8 in range(NB8):
                p = e * NB8 + b8
                wpat[t - 1, p, b8 * ST + (t - 1)] = 1.0
                if t < NPASS:
                    wpat[t - 1, p, b8 * ST + t] = -1.0
    consts["wpat"] = np.ascontiguousarray(
        wpat.transpose(1, 0, 2).reshape(128, NPASS * PCOL)).astype(bf)
    consts["identb"] = np.eye(128, dtype=f32).astype(bf)
    return consts


LAST_RESULTS = None


def _core_inputs(consts, emb_full, pred_full, c):
    import ml_dtypes
    bf = ml_dtypes.bfloat16
    bsl = slice(c * BL, (c + 1) * BL)
    m = dict(consts)
    embT = np.ascontiguousarray(
        emb_full[:, bsl, :].reshape(ROWS, D).T)          # [D, ROWS] f32
    m["emb"] = _chunk(embT, 8)
    pc = pred_full[:, bsl, :KU, :]                       # [E, 32, KU, 2]
    probs = pc[..., 0].astype(bf)
    offs_i = (pc[..., 1].astype(np.int32)
              - ST * np.arange(KU, dtype=np.int32)[None, None, :])
    # structural contract of the generator: idx = 12*k + offs, offs in [0,12)
    assert offs_i.min() >= 0 and offs_i.max() < ST, (
        "index structure violated: idx != 12*k + offs")
    offs = offs_i.astype(bf)
    def shuf(a):
        a = a.reshape(E, NBG, NB8, KU)
        return np.ascontiguousarray(
            a.transpose(1, 0, 2, 3).reshape(NBG, 128, KU))
    m["probs"] = shuf(probs)
    m["offs"] = shuf(offs)
    return m


def kernel(**inputs) -> np.ndarray:
    from concourse.bass_utils import run_bass_kernel_spmd

    inputs = {k: np.asarray(v) for k, v in inputs.items()}
    consts = _host_prep(inputs)
    use_bias = any(
        np.abs(consts[k]).max() > 0 for k in ("b1r", "b2r", "b3r"))
    nc = _program(use_bias)

    emb_full = np.asarray(inputs["endpoint_emb"], np.float32)
    pred_full = np.asarray(inputs["prediction"], np.float32)

    in_maps = [_core_inputs(consts, emb_full, pred_full, c)
               for c in range(NCORES)]

    res = run_bass_kernel_spmd(nc, in_maps, core_ids=list(range(NCORES)))
    global LAST_RESULTS
    LAST_RESULTS = res

    outf = np.zeros((B, V + 1, 2), np.float32)
    outf[:, :V, 1] = np.arange(V, dtype=np.float32)
    outf[:, V, 1] = -1.0
    for c in range(NCORES):
        # device out: [bg, kt, (b8*12+j), kk] -> [b, (kt,kk,j)]
        o = np.asarray(res.results[c]["out"], np.float32)
        o = o.reshape(NBG, NKT, NB8, ST, KT)
        o = o.transpose(0, 2, 1, 4, 3).reshape(BL, VU)
        outf[c * BL:(c + 1) * BL, :VU, 0] = o
    return outf


# revision 33
# speedup vs baseline: 1.0232x; 1.0126x over previous
"""Trainium2 Bass kernel for nn_BaseMOE (moe_routing), 8 NeuronCores.

Batch-sharded (B=256 -> 32 rows/core); full inputs in, full output out.

Per core:
  * 3-layer MLP + Wout on its [16 experts x 32 batch] rows in bf16.
    LayerNorm affine folded into the next layer's weights on the host;
    ELU via h = max(z, min(exp(z)-1, 0)); PSUM released early through an
    Activation-engine copy; LN sqrt/reciprocal on DVE so the Activation
    engine never swaps function tables; dummy matmuls warm the PE
    p-state before the MLP and through the collective gap.
  * softmax-over-batch: local exp(scores), per-expert partial sums
    exchanged with a 64-byte AllGather; all post-collective weight prep
    runs on Pool so the DVE queue (busy with threshold planes) never
    blocks on it.
  * scatter: idx[e,b,k] = 12*k + offs, offs in [0,12).  DVE builds 11
    *threshold* planes D_t = p * 1[offs < t] per [128,2048] tile with
    single tensor_mask ops (2x mode); the raw probs tile is the 12th
    plane.  TensorE recovers bucket j by linearity: output column
    (b8,j) accumulates +w*D_{j+1} - w*D_j (D_12 = p), so each (tile,j)
    costs one matmul pass and one DVE op, and the bucket difference is
    bit-exact (D planes share p's bf16 bits).  The +-w stationaries are
    Pool-built from host +-1 patterns after the collective.  Bucket
    sums [96=(b8*12+j), k] are copied to bf16 and DMA'd out; the host
    interleaves them into [B, V+1, 2] (channel 1 is a constant iota).

  All large inputs load with one DMA each (HWDGE is shared and serial,
  ~650ns per dma_start).
"""

import functools
import numpy as np

# ---- problem constants (hardcoded per contract) ----
V = 50257
E, B, K, D = 16, 256, 4097, 1024
HID = [512, 256, 128]
EPS = 1e-6
NCORES = 8
BL = B // NCORES          # 32 local batch rows per core
ST = 12                   # V // K  (index stride)
KU = K - 1                # 4096 used k slots
VU = KU * ST              # 49152 used vocab columns
NB8 = 8                   # batch rows per partition group
NBG = BL // NB8           # 4 batch groups
KT = 2048                 # k-tile
NKT = KU // KT            # 2
HK = 1024                 # half-tile k extent (PSUM half for double buffer)
PS = 512                  # psum free slice (one bank of fp32)
ROWS = E * BL             # 512 MLP rows
PCOL = NB8 * ST           # 96 = (b8, j) output columns of the e-sum matmul
NPASS = ST                # 12 moving passes per tile (D_1..D_11 + probs)
N_WARM1 = 100             # collective-gap PE warmup matmuls (keep p-state hot)


def _build_program(use_bias=False):
    from concourse import bacc
    from concourse import bass
    from concourse import tile
    import concourse.mybir as mybir

    f32 = mybir.dt.float32
    bf16 = mybir.dt.bfloat16
    AF = mybir.ActivationFunctionType
    OP = mybir.AluOpType
    X = mybir.AxisListType.X

    nc = bacc.Bacc(
        "TRN2",
        target_bir_lowering=False,
        debug=False,
        enable_asserts=False,
        num_devices=NCORES,
    )

    # ---- kernel I/O (weights pre-chunked on host: one DMA per tensor) ----
    emb = nc.declare_dram_parameter("emb", [128, 8 * ROWS], bf16, isOutput=False)
    probs_p = nc.declare_dram_parameter("probs", [NBG, 128, NKT * KT], bf16, isOutput=False)
    offs_p = nc.declare_dram_parameter("offs", [NBG, 128, NKT * KT], bf16, isOutput=False)
    w1 = nc.declare_dram_parameter("w1", [128, 8 * HID[0]], bf16, isOutput=False)
    w2 = nc.declare_dram_parameter("w2", [128, 4 * HID[1]], bf16, isOutput=False)
    w3 = nc.declare_dram_parameter("w3", [128, 2 * HID[2]], bf16, isOutput=False)
    wo = nc.declare_dram_parameter("wo", [128, 1], bf16, isOutput=False)
    b1r = nc.declare_dram_parameter("b1r", [128, HID[0]], f32, isOutput=False)
    b2r = nc.declare_dram_parameter("b2r", [128, HID[1]], f32, isOutput=False)
    b3r = nc.declare_dram_parameter("b3r", [128, HID[2]], f32, isOutput=False)
    wpat = nc.declare_dram_parameter("wpat", [128, NPASS * PCOL], bf16, isOutput=False)
    identb = nc.declare_dram_parameter("identb", [128, 128], bf16, isOutput=False)
    out = nc.declare_dram_parameter("out", [NBG, NKT, PCOL, KT], bf16, isOutput=True)

    NH = [D] + HID  # 1024, 512, 256, 128

    with tile.TileContext(nc) as tc:
        with (
            tc.tile_pool(name="const", bufs=1) as cp,
            tc.tile_pool(name="dram", bufs=1, space="DRAM") as dp,
            tc.tile_pool(name="mlp", bufs=1) as mp,
            tc.tile_pool(name="mpsum", bufs=3, space="PSUM") as mpsum,
            tc.tile_pool(name="wpsum", bufs=1, space="PSUM") as wpsum,
            tc.tile_pool(name="sc", bufs=1) as scp,
            tc.tile_pool(name="espsum", bufs=2, space="PSUM") as espsum,
        ):
            # ================= constants =================
            idb = cp.tile([128, 128], bf16, tag="idb")
            nc.sync.dma_start(out=idb[:], in_=identb[:])
            zbias = cp.tile([128, 1], f32, tag="zbias")
            nc.vector.memset(zbias[:], 0.0)
            thr = cp.tile([128, NPASS], f32, tag="thr")
            for t in range(NPASS):
                nc.vector.memset(thr[:, t:t + 1], t - 0.5)

            # few-DMA loads (emb/w1 halved so layer 1 starts sooner)
            embt = cp.tile([128, 8 * ROWS], bf16, tag="embt")
            w1t = cp.tile([128, 8 * HID[0]], bf16, tag="w1t")
            HB = 4 * ROWS
            HW1 = 4 * HID[0]
            nc.sync.dma_start(out=embt[:, :HB], in_=emb[:, :HB])
            nc.sync.dma_start(out=w1t[:, :HW1], in_=w1[:, :HW1])
            nc.sync.dma_start(out=embt[:, HB:], in_=emb[:, HB:])
            nc.sync.dma_start(out=w1t[:, HW1:], in_=w1[:, HW1:])
            w2t = cp.tile([128, 4 * HID[1]], bf16, tag="w2t")
            nc.sync.dma_start(out=w2t[:], in_=w2[:])
            w3t = cp.tile([128, 2 * HID[2]], bf16, tag="w3t")
            nc.sync.dma_start(out=w3t[:], in_=w3[:])
            wot = cp.tile([128, 1], bf16, tag="wot")
            nc.sync.dma_start(out=wot[:], in_=wo[:])
            wpatt = cp.tile([128, NPASS * PCOL], bf16, tag="wpatt")
            nc.sync.dma_start(out=wpatt[:], in_=wpat[:])

            h0T = [embt[:, c * ROWS:(c + 1) * ROWS] for c in range(8)]
            w1b = [w1t[:, c * HID[0]:(c + 1) * HID[0]] for c in range(8)]
            w2b = [w2t[:, c * HID[1]:(c + 1) * HID[1]] for c in range(4)]
            w3b = [w3t[:, c * HID[2]:(c + 1) * HID[2]] for c in range(2)]

            btiles = {1: b1r, 2: b2r, 3: b3r}
            brep = {}
            if use_bias:
                for li, dsz in ((1, HID[0]), (2, HID[1]), (3, HID[2])):
                    bt = cp.tile([128, dsz], f32, tag=f"brep{li}")
                    nc.sync.dma_start(out=bt[:], in_=btiles[li][:])
                    brep[li] = bt

            junk = cp.tile([128, PS], bf16, tag="junk")
            nc.vector.memset(junk[:], 0.0)
            warm_ps = wpsum.tile([128, PS], f32, tag="warm")

            # ================= MLP =================
            def elu_ln(psum_z, li, dsz, rc):
                """psum [128, dsz] -> normalized bf16 tile [128, dsz].

                elu(z) = max(z, min(exp(z)-1, 0)); LN via E[h]/E[h^2] so
                the stats run concurrently; normalize is one dual-ptr
                tensor_scalar.  Act only runs Exp/Square/Copy (one table).
                """
                if use_bias:
                    zb = mp.tile([128, dsz], f32, tag="eln_zb", bufs=3)
                    nc.scalar.copy(zb[:], psum_z[:])
                    nc.vector.tensor_add(zb[:], zb[:], brep[li][:])
                    zsrc = zb
                else:
                    zsrc = psum_z
                e_t = mp.tile([128, dsz], f32, tag="eln_et", bufs=3)
                nc.scalar.activation(e_t[:], zsrc[:], AF.Exp, bias=zbias[:])
                m_t = mp.tile([128, dsz], f32, tag="eln_mt", bufs=3)
                nc.vector.tensor_scalar(m_t[:], e_t[:], 1.0, 0.0, OP.subtract, OP.min)
                h = mp.tile([128, dsz], f32, tag="eln_h", bufs=3)
                s = mp.tile([128, 1], f32, tag="eln_s")
                nc.vector.tensor_tensor_reduce(
                    h[:], m_t[:], zsrc[:], 1.0, 0.0, OP.max, OP.add,
                    accum_out=s[:])
                sq = mp.tile([128, dsz], f32, tag="eln_sq", bufs=2)
                ss2 = mp.tile([128, 1], f32, tag="eln_ss2")
                nc.scalar.activation(sq[:], h[:], AF.Square, bias=zbias[:], accum_out=ss2[:])
                mu = mp.tile([128, 1], f32, tag="eln_mu")
                nc.vector.tensor_scalar(mu[:], s[:], 1.0 / dsz, None, OP.mult)
                # var = (ss2 - s^2/dsz) / (dsz-1)
                v1 = mp.tile([128, 1], f32, tag="eln_v1")
                nc.vector.tensor_scalar(v1[:], s[:], s[:], 1.0 / dsz, OP.mult, OP.mult)
                v2 = mp.tile([128, 1], f32, tag="eln_v2")
                nc.vector.tensor_tensor(v2[:], ss2[:], v1[:], OP.subtract)
                sd = mp.tile([128, 1], f32, tag="eln_sd")
                nc.vector.tensor_scalar(sd[:], v2[:], 1.0 / (dsz - 1), 0.5, OP.mult, OP.pow)
                rcp = mp.tile([128, 1], f32, tag="eln_rcp")
                nc.vector.reciprocal(rcp[:], sd[:])
                hn = mp.tile([128, dsz], bf16, tag=f"hn{li}_{rc}")
                nc.vector.tensor_scalar(hn[:], h[:], mu[:], rcp[:], OP.subtract, OP.mult)
                return hn

            def layer_mm(hT_of_rc, wtiles, rc, d_in, d_out):
                """One rc-chunk's matmuls: stationary slices from hT_of_rc."""
                pz = mpsum.tile([128, d_out], f32, tag="mt")
                nk = d_in // 128
                for fc in range(nk):
                    nc.tensor.matmul(
                        pz[:],
                        hT_of_rc[fc],
                        wtiles[fc][:],
                        start=(fc == 0),
                        stop=(fc == nk - 1),
                    )
                return pz

            def trans_rc(h_rc, d_feat, name, rc):
                """h_rc [128 rows, d_feat] -> per-fc [128 feat, 128 rows] slices."""
                t = mp.tile([128, (d_feat // 128) * 128], bf16, tag=f"{name}T{rc}")
                outs = []
                for fc in range(d_feat // 128):
                    pt = mpsum.tile([128, 128], bf16, tag="mt")
                    nc.tensor.transpose(
                        pt[:], h_rc[:, fc * 128:(fc + 1) * 128], idb[:])
                    sl = t[:, fc * 128:(fc + 1) * 128]
                    nc.scalar.copy(sl, pt[:])
                    outs.append(sl)
                return outs

            # layer 1 (stationary = pre-transposed emb chunks)
            h1 = []
            for rc in range(4):
                hT_rc = [h0T[fc][:, rc * 128:(rc + 1) * 128] for fc in range(8)]
                pz = layer_mm(hT_rc, w1b, rc, NH[0], NH[1])
                h1.append(elu_ln(pz, 1, NH[1], rc))
            # layers 2/3 pipelined per rc-chunk through transpose; a few
            # gap-filler warmups keep the p-state ramp alive across the
            # eln-latency bubbles at layer transitions
            h2 = []
            for rc in range(4):
                hT_rc = trans_rc(h1[rc][:], NH[1], "h1", rc)
                pz = layer_mm(hT_rc, w2b, rc, NH[1], NH[2])
                h2.append(elu_ln(pz, 2, NH[2], rc))
            h3 = []
            for rc in range(4):
                hT_rc = trans_rc(h2[rc][:], NH[2], "h2", rc)
                pz = layer_mm(hT_rc, w3b, rc, NH[2], NH[3])
                h3.append(elu_ln(pz, 3, NH[3], rc))
            # scores need h3T[fc=0] spanning all rc
            h3T = mp.tile([128, ROWS], bf16, tag="h3T")
            for rc in range(4):
                pt = mpsum.tile([128, 128], bf16, tag="mt")
                nc.tensor.transpose(pt[:], h3[rc][:, :128], idb[:])
                nc.scalar.copy(h3T[:, rc * 128:(rc + 1) * 128], pt[:])

            ps_s = mpsum.tile([1, ROWS], f32, tag="mt")
            nc.tensor.matmul(ps_s[:], wot[:], h3T[:], start=True, stop=True)
            scores = mp.tile([1, ROWS], f32, tag="scores")
            nc.scalar.copy(scores[:], ps_s[:])

            # gate the threshold planes on MLP completion: a zero tile that
            # data-depends on h3 keeps DVE free for the eln chains until the
            # MLP is done (the scheduler would otherwise starve them behind
            # ready mask ops)
            z128 = mp.tile([128, 1], f32, tag="z128")
            nc.scalar.activation(z128[:], h3[3][:, :1], AF.Copy, bias=0.0,
                                 scale=0.0)
            thrf = cp.tile([128, NPASS], f32, tag="thrf")
            nc.vector.tensor_scalar(thrf[:], thr[:], z128[:], None, OP.add)
            # warmups gated on MLP completion so they span the collective
            # window instead of padding the MLP's PE stream
            junk2 = cp.tile([128, PS], bf16, tag="junk2")
            nc.scalar.activation(junk2[:], junk[:], AF.Relu, bias=z128[:])

            # ---- softmax over full batch, DVE-free (DVE is busy with
            # threshold planes): scores -> [E, BL] partitions, Act exp
            # with accum gives both w16 and the per-expert partials ----
            s2 = mp.tile([E, BL], f32, tag="s2")
            nc.sync.dma_start(out=s2[:], in_=scores[:1, :])
            w16 = mp.tile([E, BL], f32, tag="w16")
            smy16 = mp.tile([E, 1], f32, tag="smy16")
            nc.scalar.activation(w16[:], s2[:], AF.Exp, bias=zbias[:E, :],
                                 accum_out=smy16[:])
            cc_in = dp.tile([E, 1], f32, tag="ccin")
            cc_out = dp.tile([NCORES, E], f32, tag="ccout")
            nc.sync.dma_start(out=cc_in[:], in_=smy16[:])
            nc.gpsimd.collective_compute(
                "AllGather",
                OP.bypass,
                replica_groups=[list(range(NCORES))],
                ins=[cc_in[:].opt()],
                outs=[cc_out[:].opt()],
            )
            # numerator-weighted stationaries build BEFORE the collective on
            # the Activation engine (scale is a per-partition pointer); only
            # the tiny 1/denominator pass stays on the post-collective path
            w16_pp = cp.tile([128, NBG], f32, tag="w16pp")
            for bg in range(NBG):
                nc.sync.dma_start(
                    out=w16_pp[:, bg:bg + 1],
                    in_=w16[:, bg * NB8:(bg + 1) * NB8],
                )
            statu = [[None] * (NPASS + 1) for _ in range(NBG)]
            for bg in range(NBG):
                for t in range(1, NPASS + 1):
                    su = cp.tile([128, PCOL], bf16, tag=f"statu{bg}_{t}")
                    nc.scalar.activation(
                        su[:], wpatt[:, (t - 1) * PCOL:t * PCOL], AF.Identity,
                        scale=w16_pp[:, bg:bg + 1])
                    statu[bg][t] = su
            # gathered partials, transposed to [E, NCORES]; Act accum sums
            sgT = mp.tile([E, NCORES], f32, tag="sgT")
            nc.sync.dma_start(out=sgT[:], in_=cc_out[:].rearrange("c e -> e c"))
            sgc = mp.tile([E, NCORES], f32, tag="sgc")
            s16 = mp.tile([E, 1], f32, tag="s16")
            nc.scalar.activation(sgc[:], sgT[:], AF.Copy, bias=0.0,
                                 accum_out=s16[:])
            rcp16 = mp.tile([E, 1], f32, tag="rcp16")
            nc.gpsimd.tensor_scalar(rcp16[:], s16[:], -1.0, None, OP.pow)
            ones8 = cp.tile([E, NB8], f32, tag="ones8")
            nc.vector.memset(ones8[:], 1.0)
            rcpw = mp.tile([E, NB8], f32, tag="rcpw")
            nc.gpsimd.tensor_scalar(rcpw[:], ones8[:], rcp16[:], None, OP.mult)
            rcp_pp = cp.tile([128, 1], f32, tag="rcppp")
            nc.sync.dma_start(out=rcp_pp[:], in_=rcpw[:])
            # final stationaries = statu * (1/denom), on Pool, bg-major
            stat = [[None] * (NPASS + 1) for _ in range(NBG)]
            for bg in range(NBG):
                for t in range(1, NPASS + 1):
                    st_t = cp.tile([128, PCOL], bf16, tag=f"stat{bg}_{t}")
                    nc.gpsimd.tensor_scalar(
                        st_t[:], statu[bg][t][:], rcp_pp[:], None, OP.mult)
                    stat[bg][t] = st_t

            # keep the PE p-state hot through the collective gap
            for _ in range(N_WARM1):
                nc.tensor.matmul(warm_ps[:], idb[:], junk2[:],
                                 start=True, stop=True, skip_group_check=True)

            # ================= scatter =================
            # D_t = p * 1[offs < t] for t=1..11; pass 12 = raw probs.
            # Column (b8,j) of es accumulates +w*D_{j+1} - w*D_j.
            for bg in range(NBG):
                for kt in range(NKT):
                    prb = scp.tile([128, KT], bf16, tag="prb", bufs=3)
                    nc.sync.dma_start(out=prb[:], in_=probs_p[bg, :, kt * KT:(kt + 1) * KT])
                    ofs = scp.tile([128, KT], bf16, tag="ofs", bufs=3)
                    nc.sync.dma_start(out=ofs[:], in_=offs_p[bg, :, kt * KT:(kt + 1) * KT])
                    planes = [None] * (NPASS + 1)
                    for t in range(1, NPASS - 1):
                        d_t = scp.tile([128, KT], bf16, tag=f"D{t}", bufs=2)
                        nc.vector.tensor_mask(
                            d_t[:], prb[:], thrf[:, t:t + 1], ofs[:], 0)
                        planes[t] = d_t[:]
                    # D_11 on the (post-collective idle) Pool engine
                    t = NPASS - 1
                    mskp = scp.tile([128, KT], bf16, tag="mskp", bufs=1)
                    nc.gpsimd.tensor_scalar(
                        mskp[:], ofs[:], t - 0.5, None, OP.is_lt)
                    d11 = scp.tile([128, KT], bf16, tag=f"D{t}", bufs=2)
                    nc.gpsimd.tensor_tensor(
                        d11[:], mskp[:], prb[:], OP.mult)
                    planes[t] = d11[:]
                    planes[NPASS] = prb[:]
                    ob = scp.tile([PCOL, KT], bf16, tag="ob", bufs=2)
                    for h in range(2):
                        es = espsum.tile([PCOL, HK], f32, tag="es")
                        for t in range(1, NPASS + 1):
                            for s in range(HK // PS):
                                lo = h * HK + s * PS
                                nc.tensor.matmul(
                                    es[:, s * PS:(s + 1) * PS],
                                    stat[bg][t][:],
                                    planes[t][:, lo:lo + PS],
                                    start=(t == 1),
                                    stop=(t == NPASS),
                                    skip_group_check=True,
                                )
                        nc.scalar.copy(ob[:, h * HK:(h + 1) * HK], es[:])
                    nc.sync.dma_start(out=out[bg, kt], in_=ob[:])
    nc.compile()
    return nc


@functools.lru_cache(maxsize=2)
def _program(use_bias=False):
    return _build_program(use_bias)


def _chunk(a, nch):
    """[nch*128, dout] f32 -> [128, nch*dout] bf16 (chunk-major free)."""
    import ml_dtypes
    dout = a.shape[1]
    return np.ascontiguousarray(
        a.reshape(nch, 128, dout).transpose(1, 0, 2).reshape(128, nch * dout)
    ).astype(ml_dtypes.bfloat16)


def _host_prep(inputs):
    """Fold LN affine params into following layers; build constants."""
    import ml_dtypes
    f32 = np.float32
    bf = ml_dtypes.bfloat16
    W1 = inputs["W1"].astype(np.float64)
    W2 = inputs["W2"].astype(np.float64)
    W3 = inputs["W3"].astype(np.float64)
    Wout = inputs["Wout"].astype(np.float64)
    g1, be1 = inputs["g1"].astype(np.float64), inputs["be1"].astype(np.float64)
    g2, be2 = inputs["g2"].astype(np.float64), inputs["be2"].astype(np.float64)
    g3 = inputs["g3"].astype(np.float64)
    b1, b2, b3 = (inputs["b1"].astype(np.float64), inputs["b2"].astype(np.float64),
                  inputs["b3"].astype(np.float64))

    w1f = W1
    b1f = b1
    w2f = g1[:, None] * W2
    b2f = b2 + be1 @ W2
    w3f = g2[:, None] * W3
    b3f = b3 + be2 @ W3
    wof = g3[:, None] * Wout
    # bout / be3@Wout shift all scores equally -> softmax-invariant, dropped.

    consts = {
        "w1": _chunk(w1f.astype(f32), 8),
        "w2": _chunk(w2f.astype(f32), 4),
        "w3": _chunk(w3f.astype(f32), 2),
        "wo": wof.astype(f32).astype(bf),
        "b1r": np.broadcast_to(b1f.astype(f32), (128, HID[0])).copy(),
        "b2r": np.broadcast_to(b2f.astype(f32), (128, HID[1])).copy(),
        "b3r": np.broadcast_to(b3f.astype(f32), (128, HID[2])).copy(),
    }

    # +-1 patterns: pass t feeds column (b8, t-1) with +1 and column
    # (b8, t) with -1 (pass 12 = raw probs only feeds column 11).
    wpat = np.zeros((NPASS, 128, PCOL), f32)
    for t in range(1, NPASS + 1):
        for e in range(E):
            for b8 in range(NB8):
                p = e * NB8 + b8
                wpat[t - 1, p, b8 * ST + (t - 1)] = 1.0
                if t < NPASS:
                    wpat[t - 1, p, b8 * ST + t] = -1.0
    consts["wpat"] = np.ascontiguousarray(
        wpat.transpose(1, 0, 2).reshape(128, NPASS * PCOL)).astype(bf)
    consts["identb"] = np.eye(128, dtype=f32).astype(bf)
    return consts


LAST_RESULTS = None


def _core_inputs(consts, emb_full, pred_full, c):
    import ml_dtypes
    bf = ml_dtypes.bfloat16
    bsl = slice(c * BL, (c + 1) * BL)
    m = dict(consts)
    embT = np.ascontiguousarray(
        emb_full[:, bsl, :].reshape(ROWS, D).T)          # [D, ROWS] f32
    m["emb"] = _chunk(embT, 8)
    pc = pred_full[:, bsl, :KU, :]                       # [E, 32, KU, 2]
    probs = pc[..., 0].astype(bf)
    offs_i = (pc[..., 1].astype(np.int32)
              - ST * np.arange(KU, dtype=np.int32)[None, None, :])
    # structural contract of the generator: idx = 12*k + offs, offs in [0,12)
    assert offs_i.min() >= 0 and offs_i.max() < ST, (
        "index structure violated: idx != 12*k + offs")
    offs = offs_i.astype(bf)
    def shuf(a):
        a = a.reshape(E, NBG, NB8, KU)
        return np.ascontiguousarray(
            a.transpose(1, 0, 2, 3).reshape(NBG, 128, KU))
    m["probs"] = shuf(probs)
    m["offs"] = shuf(offs)
    return m


def kernel(**inputs) -> np.ndarray:
    from concourse.bass_utils import run_bass_kernel_spmd

    inputs = {k: np.asarray(v) for k, v in inputs.items()}
    consts = _host_prep(inputs)
    use_bias = any(
        np.abs(consts[k]).max() > 0 for k in ("b1r", "b2r", "b3r"))
    nc = _program(use_bias)

    emb_full = np.asarray(inputs["endpoint_emb"], np.float32)
    pred_full = np.asarray(inputs["prediction"], np.float32)

    in_maps = [_core_inputs(consts, emb_full, pred_full, c)
               for c in range(NCORES)]

    res = run_bass_kernel_spmd(nc, in_maps, core_ids=list(range(NCORES)))
    global LAST_RESULTS
    LAST_RESULTS = res

    outf = np.zeros((B, V + 1, 2), np.float32)
    outf[:, :V, 1] = np.arange(V, dtype=np.float32)
    outf[:, V, 1] = -1.0
    for c in range(NCORES):
        # device out: [bg, kt, (b8*12+j), kk] -> [b, (kt,kk,j)]
        o = np.asarray(res.results[c]["out"], np.float32)
        o = o.reshape(NBG, NKT, NB8, ST, KT)
        o = o.transpose(0, 2, 1, 4, 3).reshape(BL, VU)
        outf[c * BL:(c + 1) * BL, :VU, 0] = o
    return outf


# revision 38
# speedup vs baseline: 1.0853x; 1.0607x over previous
"""Trainium2 Bass kernel for nn_BaseMOE (moe_routing), 8 NeuronCores.

Batch-sharded (B=256 -> 32 rows/core); full inputs in, full output out.

Per core:
  * 3-layer MLP + Wout on its [16 experts x 32 batch] rows in bf16.
    LayerNorm affine folded into the next layer's weights on the host;
    ELU via h = max(z, min(exp(z)-1, 0)); PSUM released early through an
    Activation-engine copy; LN sqrt/reciprocal on DVE so the Activation
    engine never swaps function tables; dummy matmuls warm the PE
    p-state before the MLP and through the collective gap.
  * softmax-over-batch: local exp(scores), per-expert partial sums
    exchanged with a 64-byte AllGather; all post-collective weight prep
    runs on Pool so the DVE queue (busy with threshold planes) never
    blocks on it.
  * scatter: idx[e,b,k] = 12*k + offs, offs in [0,12).  DVE builds 11
    *threshold* planes D_t = p * 1[offs < t] per [128,2048] tile with
    single tensor_mask ops (2x mode); the raw probs tile is the 12th
    plane.  TensorE recovers bucket j by linearity: output column
    (b8,j) accumulates +w*D_{j+1} - w*D_j (D_12 = p), so each (tile,j)
    costs one matmul pass and one DVE op, and the bucket difference is
    bit-exact (D planes share p's bf16 bits).  The +-w stationaries are
    Pool-built from host +-1 patterns after the collective.  Bucket
    sums [96=(b8*12+j), k] are copied to bf16 and DMA'd out; the host
    interleaves them into [B, V+1, 2] (channel 1 is a constant iota).

  All large inputs load with one DMA each (HWDGE is shared and serial,
  ~650ns per dma_start).
"""

import functools
import numpy as np

# ---- problem constants (hardcoded per contract) ----
V = 50257
E, B, K, D = 16, 256, 4097, 1024
HID = [512, 256, 128]
EPS = 1e-6
NCORES = 8
BL = B // NCORES          # 32 local batch rows per core
ST = 12                   # V // K  (index stride)
KU = K - 1                # 4096 used k slots
VU = KU * ST              # 49152 used vocab columns
NB8 = 8                   # batch rows per partition group
NBG = BL // NB8           # 4 batch groups
KT = 2048                 # k-tile
NKT = KU // KT            # 2
HK = 1024                 # half-tile k extent (PSUM half for double buffer)
PS = 512                  # psum free slice (one bank of fp32)
ROWS = E * BL             # 512 MLP rows
PCOL = NB8 * ST           # 96 = (b8, j) output columns of the e-sum matmul
NPASS = ST                # 12 moving passes per tile (D_1..D_11 + probs)
N_WARM1 = 100             # collective-gap PE warmup matmuls (keep p-state hot)


def _build_program(use_bias=False):
    from concourse import bacc
    from concourse import bass
    from concourse import tile
    import concourse.mybir as mybir

    f32 = mybir.dt.float32
    bf16 = mybir.dt.bfloat16
    AF = mybir.ActivationFunctionType
    OP = mybir.AluOpType
    X = mybir.AxisListType.X

    nc = bacc.Bacc(
        "TRN2",
        target_bir_lowering=False,
        debug=False,
        enable_asserts=False,
        num_devices=NCORES,
    )

    # ---- kernel I/O (weights pre-chunked on host: one DMA per tensor) ----
    emb = nc.declare_dram_parameter("emb", [128, 8 * ROWS], bf16, isOutput=False)
    probs_p = nc.declare_dram_parameter("probs", [NBG, 128, NKT * KT], bf16, isOutput=False)
    offs_p = nc.declare_dram_parameter("offs", [NBG, 128, NKT * KT], bf16, isOutput=False)
    w1 = nc.declare_dram_parameter("w1", [128, 8 * HID[0]], bf16, isOutput=False)
    w2 = nc.declare_dram_parameter("w2", [128, 4 * HID[1]], bf16, isOutput=False)
    w3 = nc.declare_dram_parameter("w3", [128, 2 * HID[2]], bf16, isOutput=False)
    wo = nc.declare_dram_parameter("wo", [128, 1], bf16, isOutput=False)
    b1r = nc.declare_dram_parameter("b1r", [128, HID[0]], f32, isOutput=False)
    b2r = nc.declare_dram_parameter("b2r", [128, HID[1]], f32, isOutput=False)
    b3r = nc.declare_dram_parameter("b3r", [128, HID[2]], f32, isOutput=False)
    wpat = nc.declare_dram_parameter("wpat", [128, NPASS * PCOL], bf16, isOutput=False)
    identb = nc.declare_dram_parameter("identb", [128, 128], bf16, isOutput=False)
    out = nc.declare_dram_parameter("out", [NBG, NKT, PCOL, KT], bf16, isOutput=True)

    NH = [D] + HID  # 1024, 512, 256, 128

    with tile.TileContext(nc) as tc:
        with (
            tc.tile_pool(name="const", bufs=1) as cp,
            tc.tile_pool(name="dram", bufs=1, space="DRAM") as dp,
            tc.tile_pool(name="mlp", bufs=1) as mp,
            tc.tile_pool(name="mpsum", bufs=3, space="PSUM") as mpsum,
            tc.tile_pool(name="wpsum", bufs=1, space="PSUM") as wpsum,
            tc.tile_pool(name="sc", bufs=1) as scp,
            tc.tile_pool(name="espsum", bufs=2, space="PSUM") as espsum,
        ):
            # ================= constants =================
            idb = cp.tile([128, 128], bf16, tag="idb")
            nc.sync.dma_start(out=idb[:], in_=identb[:])
            zbias = cp.tile([128, 1], f32, tag="zbias")
            nc.vector.memset(zbias[:], 0.0)
            thr = cp.tile([128, NPASS], f32, tag="thr")
            for t in range(NPASS):
                nc.vector.memset(thr[:, t:t + 1], t - 0.5)

            # few-DMA loads (emb/w1 halved so layer 1 starts sooner)
            embt = cp.tile([128, 8 * ROWS], bf16, tag="embt")
            w1t = cp.tile([128, 8 * HID[0]], bf16, tag="w1t")
            HB = 4 * ROWS
            HW1 = 4 * HID[0]
            nc.sync.dma_start(out=embt[:, :HB], in_=emb[:, :HB])
            nc.sync.dma_start(out=w1t[:, :HW1], in_=w1[:, :HW1])
            nc.sync.dma_start(out=embt[:, HB:], in_=emb[:, HB:])
            nc.sync.dma_start(out=w1t[:, HW1:], in_=w1[:, HW1:])
            w2t = cp.tile([128, 4 * HID[1]], bf16, tag="w2t")
            nc.sync.dma_start(out=w2t[:], in_=w2[:])
            w3t = cp.tile([128, 2 * HID[2]], bf16, tag="w3t")
            nc.sync.dma_start(out=w3t[:], in_=w3[:])
            wot = cp.tile([128, 1], bf16, tag="wot")
            nc.sync.dma_start(out=wot[:], in_=wo[:])
            wpatt = cp.tile([128, NPASS * PCOL], bf16, tag="wpatt")
            nc.sync.dma_start(out=wpatt[:], in_=wpat[:])

            h0T = [embt[:, c * ROWS:(c + 1) * ROWS] for c in range(8)]
            w1b = [w1t[:, c * HID[0]:(c + 1) * HID[0]] for c in range(8)]
            w2b = [w2t[:, c * HID[1]:(c + 1) * HID[1]] for c in range(4)]
            w3b = [w3t[:, c * HID[2]:(c + 1) * HID[2]] for c in range(2)]

            btiles = {1: b1r, 2: b2r, 3: b3r}
            brep = {}
            if use_bias:
                for li, dsz in ((1, HID[0]), (2, HID[1]), (3, HID[2])):
                    bt = cp.tile([128, dsz], f32, tag=f"brep{li}")
                    nc.sync.dma_start(out=bt[:], in_=btiles[li][:])
                    brep[li] = bt

            junk = cp.tile([128, PS], bf16, tag="junk")
            nc.vector.memset(junk[:], 0.0)
            warm_ps = wpsum.tile([128, PS], f32, tag="warm")

            # ================= MLP =================
            def elu_ln(psum_z, li, dsz, rc):
                """psum [128, dsz] -> normalized bf16 tile [128, dsz].

                elu(z) = max(z, min(exp(z)-1, 0)); LN via E[h]/E[h^2] so
                the stats run concurrently; normalize is one dual-ptr
                tensor_scalar.  Act only runs Exp/Square/Copy (one table).
                """
                if use_bias:
                    zb = mp.tile([128, dsz], f32, tag="eln_zb", bufs=3)
                    nc.scalar.copy(zb[:], psum_z[:])
                    nc.vector.tensor_add(zb[:], zb[:], brep[li][:])
                    zsrc = zb
                else:
                    zsrc = psum_z
                e_t = mp.tile([128, dsz], f32, tag="eln_et", bufs=3)
                nc.scalar.activation(e_t[:], zsrc[:], AF.Exp, bias=zbias[:])
                m_t = mp.tile([128, dsz], f32, tag="eln_mt", bufs=3)
                nc.vector.tensor_scalar(m_t[:], e_t[:], 1.0, 0.0, OP.subtract, OP.min)
                h = mp.tile([128, dsz], f32, tag="eln_h", bufs=3)
                s = mp.tile([128, 1], f32, tag="eln_s")
                nc.vector.tensor_tensor_reduce(
                    h[:], m_t[:], zsrc[:], 1.0, 0.0, OP.max, OP.add,
                    accum_out=s[:])
                sq = mp.tile([128, dsz], f32, tag="eln_sq", bufs=2)
                ss2 = mp.tile([128, 1], f32, tag="eln_ss2")
                nc.scalar.activation(sq[:], h[:], AF.Square, bias=zbias[:], accum_out=ss2[:])
                mu = mp.tile([128, 1], f32, tag="eln_mu")
                nc.vector.tensor_scalar(mu[:], s[:], 1.0 / dsz, None, OP.mult)
                # var = (ss2 - s^2/dsz) / (dsz-1)
                v1 = mp.tile([128, 1], f32, tag="eln_v1")
                nc.vector.tensor_scalar(v1[:], s[:], s[:], 1.0 / dsz, OP.mult, OP.mult)
                v2 = mp.tile([128, 1], f32, tag="eln_v2")
                nc.vector.tensor_tensor(v2[:], ss2[:], v1[:], OP.subtract)
                sd = mp.tile([128, 1], f32, tag="eln_sd")
                nc.vector.tensor_scalar(sd[:], v2[:], 1.0 / (dsz - 1), 0.5, OP.mult, OP.pow)
                rcp = mp.tile([128, 1], f32, tag="eln_rcp")
                nc.vector.reciprocal(rcp[:], sd[:])
                hn = mp.tile([128, dsz], bf16, tag=f"hn{li}_{rc}")
                nc.vector.tensor_scalar(hn[:], h[:], mu[:], rcp[:], OP.subtract, OP.mult)
                return hn

            def layer_mm(hT_of_rc, wtiles, rc, d_in, d_out):
                """One rc-chunk's matmuls: stationary slices from hT_of_rc."""
                pz = mpsum.tile([128, d_out], f32, tag="mt")
                nk = d_in // 128
                for fc in range(nk):
                    nc.tensor.matmul(
                        pz[:],
                        hT_of_rc[fc],
                        wtiles[fc][:],
                        start=(fc == 0),
                        stop=(fc == nk - 1),
                    )
                return pz

            def trans_rc(h_rc, d_feat, name, rc):
                """h_rc [128 rows, d_feat] -> per-fc [128 feat, 128 rows] slices."""
                t = mp.tile([128, (d_feat // 128) * 128], bf16, tag=f"{name}T{rc}")
                outs = []
                for fc in range(d_feat // 128):
                    pt = mpsum.tile([128, 128], bf16, tag="mt")
                    nc.tensor.transpose(
                        pt[:], h_rc[:, fc * 128:(fc + 1) * 128], idb[:])
                    sl = t[:, fc * 128:(fc + 1) * 128]
                    nc.scalar.copy(sl, pt[:])
                    outs.append(sl)
                return outs

            # layer 1 (stationary = pre-transposed emb chunks)
            h1 = []
            for rc in range(4):
                hT_rc = [h0T[fc][:, rc * 128:(rc + 1) * 128] for fc in range(8)]
                pz = layer_mm(hT_rc, w1b, rc, NH[0], NH[1])
                h1.append(elu_ln(pz, 1, NH[1], rc))
            # layers 2/3 pipelined per rc-chunk through transpose; a few
            # gap-filler warmups keep the p-state ramp alive across the
            # eln-latency bubbles at layer transitions
            h2 = []
            for rc in range(4):
                hT_rc = trans_rc(h1[rc][:], NH[1], "h1", rc)
                pz = layer_mm(hT_rc, w2b, rc, NH[1], NH[2])
                h2.append(elu_ln(pz, 2, NH[2], rc))
            h3 = []
            for rc in range(4):
                hT_rc = trans_rc(h2[rc][:], NH[2], "h2", rc)
                pz = layer_mm(hT_rc, w3b, rc, NH[2], NH[3])
                h3.append(elu_ln(pz, 3, NH[3], rc))
            # scores need h3T[fc=0] spanning all rc
            h3T = mp.tile([128, ROWS], bf16, tag="h3T")
            for rc in range(4):
                pt = mpsum.tile([128, 128], bf16, tag="mt")
                nc.tensor.transpose(pt[:], h3[rc][:, :128], idb[:])
                nc.scalar.copy(h3T[:, rc * 128:(rc + 1) * 128], pt[:])

            ps_s = mpsum.tile([1, ROWS], f32, tag="mt")
            nc.tensor.matmul(ps_s[:], wot[:], h3T[:], start=True, stop=True)
            scores = mp.tile([1, ROWS], f32, tag="scores")
            nc.scalar.copy(scores[:], ps_s[:])

            # gate the threshold planes on MLP completion: a zero tile that
            # data-depends on h3 keeps DVE free for the eln chains until the
            # MLP is done (the scheduler would otherwise starve them behind
            # ready mask ops)
            z128 = mp.tile([128, 1], f32, tag="z128")
            nc.scalar.activation(z128[:], h3[3][:, :1], AF.Copy, bias=0.0,
                                 scale=0.0)
            thrf = cp.tile([128, NPASS], f32, tag="thrf")
            nc.vector.tensor_scalar(thrf[:], thr[:], z128[:], None, OP.add)
            # warmups gated on MLP completion so they span the collective
            # window instead of padding the MLP's PE stream
            junk2 = cp.tile([128, PS], bf16, tag="junk2")
            nc.scalar.activation(junk2[:], junk[:], AF.Relu, bias=z128[:])

            # ---- softmax over full batch, DVE-free (DVE is busy with
            # threshold planes): scores -> [E, BL] partitions, Act exp
            # with accum gives both w16 and the per-expert partials ----
            s2 = mp.tile([E, BL], f32, tag="s2")
            nc.scalar.dma_start(out=s2[:], in_=scores[:1, :])
            w16 = mp.tile([E, BL], f32, tag="w16")
            smy16 = mp.tile([E, 1], f32, tag="smy16")
            nc.scalar.activation(w16[:], s2[:], AF.Exp, bias=zbias[:E, :],
                                 accum_out=smy16[:])
            cc_in = dp.tile([E, 1], f32, tag="ccin")
            cc_out = dp.tile([NCORES, E], f32, tag="ccout")
            nc.scalar.dma_start(out=cc_in[:], in_=smy16[:])
            nc.gpsimd.collective_compute(
                "AllGather",
                OP.bypass,
                replica_groups=[list(range(NCORES))],
                ins=[cc_in[:].opt()],
                outs=[cc_out[:].opt()],
            )
            # numerator-weighted stationaries build BEFORE the collective on
            # the Activation engine (scale is a per-partition pointer); only
            # the tiny 1/denominator pass stays on the post-collective path
            w16_pp = cp.tile([128, NBG], f32, tag="w16pp")
            for bg in range(NBG):
                nc.sync.dma_start(
                    out=w16_pp[:, bg:bg + 1],
                    in_=w16[:, bg * NB8:(bg + 1) * NB8],
                )
            statu = [[None] * (NPASS + 1) for _ in range(NBG)]
            for bg in range(NBG):
                for t in range(1, NPASS + 1):
                    su = cp.tile([128, PCOL], bf16, tag=f"statu{bg}_{t}")
                    nc.scalar.activation(
                        su[:], wpatt[:, (t - 1) * PCOL:t * PCOL], AF.Identity,
                        scale=w16_pp[:, bg:bg + 1])
                    statu[bg][t] = su
            # gathered partials, transposed to [E, NCORES]; Act accum sums
            sgT = mp.tile([E, NCORES], f32, tag="sgT")
            nc.scalar.dma_start(out=sgT[:], in_=cc_out[:].rearrange("c e -> e c"))
            sgc = mp.tile([E, NCORES], f32, tag="sgc")
            s16 = mp.tile([E, 1], f32, tag="s16")
            nc.scalar.activation(sgc[:], sgT[:], AF.Copy, bias=0.0,
                                 accum_out=s16[:])
            rcp16 = mp.tile([E, 1], f32, tag="rcp16")
            nc.gpsimd.tensor_scalar(rcp16[:], s16[:], -1.0, None, OP.pow)
            ones8 = cp.tile([E, NB8], f32, tag="ones8")
            nc.vector.memset(ones8[:], 1.0)
            rcpw = mp.tile([E, NB8], f32, tag="rcpw")
            nc.gpsimd.tensor_scalar(rcpw[:], ones8[:], rcp16[:], None, OP.mult)
            rcp_pp = cp.tile([128, 1], f32, tag="rcppp")
            nc.gpsimd.dma_start(out=rcp_pp[:], in_=rcpw[:])
            # final stationaries = statu * (1/denom), on Pool, bg-major
            stat = [[None] * (NPASS + 1) for _ in range(NBG)]
            for bg in range(NBG):
                for t in range(1, NPASS + 1):
                    st_t = cp.tile([128, PCOL], bf16, tag=f"stat{bg}_{t}")
                    nc.gpsimd.tensor_scalar(
                        st_t[:], statu[bg][t][:], rcp_pp[:], None, OP.mult)
                    stat[bg][t] = st_t

            # keep the PE p-state hot through the collective gap
            for _ in range(N_WARM1):
                nc.tensor.matmul(warm_ps[:], idb[:], junk2[:],
                                 start=True, stop=True, skip_group_check=True)

            # ================= scatter =================
            # D_t = p * 1[offs < t] for t=1..11; pass 12 = raw probs.
            # Column (b8,j) of es accumulates +w*D_{j+1} - w*D_j.
            for bg in range(NBG):
                for kt in range(NKT):
                    prb = scp.tile([128, KT], bf16, tag="prb", bufs=3)
                    nc.sync.dma_start(out=prb[:], in_=probs_p[bg, :, kt * KT:(kt + 1) * KT])
                    ofs = scp.tile([128, KT], bf16, tag="ofs", bufs=3)
                    nc.sync.dma_start(out=ofs[:], in_=offs_p[bg, :, kt * KT:(kt + 1) * KT])
                    planes = [None] * (NPASS + 1)
                    for t in range(1, NPASS - 2):
                        d_t = scp.tile([128, KT], bf16, tag=f"D{t}", bufs=2)
                        nc.vector.tensor_mask(
                            d_t[:], prb[:], thrf[:, t:t + 1], ofs[:], 0)
                        planes[t] = d_t[:]
                    # D_10 split: first half DVE, second half Pool; D_11
                    # fully on Pool (both engines have slack vs DVE)
                    t = NPASS - 2
                    d10 = scp.tile([128, KT], bf16, tag=f"D{t}", bufs=2)
                    nc.vector.tensor_mask(
                        d10[:, :HK], prb[:, :HK], thrf[:, t:t + 1],
                        ofs[:, :HK], 0)
                    mskq = scp.tile([128, HK], bf16, tag="mskq", bufs=1)
                    nc.gpsimd.tensor_scalar(
                        mskq[:], ofs[:, HK:], t - 0.5, None, OP.is_lt)
                    nc.gpsimd.tensor_tensor(
                        d10[:, HK:], mskq[:], prb[:, HK:], OP.mult)
                    planes[t] = d10[:]
                    t = NPASS - 1
                    mskp = scp.tile([128, KT], bf16, tag="mskp", bufs=1)
                    nc.gpsimd.tensor_scalar(
                        mskp[:], ofs[:], t - 0.5, None, OP.is_lt)
                    d11 = scp.tile([128, KT], bf16, tag=f"D{t}", bufs=2)
                    nc.gpsimd.tensor_tensor(
                        d11[:], mskp[:], prb[:], OP.mult)
                    planes[t] = d11[:]
                    planes[NPASS] = prb[:]
                    ob = scp.tile([PCOL, KT], bf16, tag="ob", bufs=2)
                    for h in range(2):
                        es = espsum.tile([PCOL, HK], f32, tag="es")
                        for t in range(1, NPASS + 1):
                            for s in range(HK // PS):
                                lo = h * HK + s * PS
                                nc.tensor.matmul(
                                    es[:, s * PS:(s + 1) * PS],
                                    stat[bg][t][:],
                                    planes[t][:, lo:lo + PS],
                                    start=(t == 1),
                                    stop=(t == NPASS),
                                    skip_group_check=True,
                                )
                        nc.scalar.copy(ob[:, h * HK:(h + 1) * HK], es[:])
                    nc.sync.dma_start(out=out[bg, kt], in_=ob[:])
    nc.compile()
    return nc


@functools.lru_cache(maxsize=2)
def _program(use_bias=False):
    return _build_program(use_bias)


def _chunk(a, nch):
    """[nch*128, dout] f32 -> [128, nch*dout] bf16 (chunk-major free)."""
    import ml_dtypes
    dout = a.shape[1]
    return np.ascontiguousarray(
        a.reshape(nch, 128, dout).transpose(1, 0, 2).reshape(128, nch * dout)
    ).astype(ml_dtypes.bfloat16)


def _host_prep(inputs):
    """Fold LN affine params into following layers; build constants."""
    import ml_dtypes
    f32 = np.float32
    bf = ml_dtypes.bfloat16
    W1 = inputs["W1"].astype(np.float64)
    W2 = inputs["W2"].astype(np.float64)
    W3 = inputs["W3"].astype(np.float64)
    Wout = inputs["Wout"].astype(np.float64)
    g1, be1 = inputs["g1"].astype(np.float64), inputs["be1"].astype(np.float64)
    g2, be2 = inputs["g2"].astype(np.float64), inputs["be2"].astype(np.float64)
    g3 = inputs["g3"].astype(np.float64)
    b1, b2, b3 = (inputs["b1"].astype(np.float64), inputs["b2"].astype(np.float64),
                  inputs["b3"].astype(np.float64))

    w1f = W1
    b1f = b1
    w2f = g1[:, None] * W2
    b2f = b2 + be1 @ W2
    w3f = g2[:, None] * W3
    b3f = b3 + be2 @ W3
    wof = g3[:, None] * Wout
    # bout / be3@Wout shift all scores equally -> softmax-invariant, dropped.

    consts = {
        "w1": _chunk(w1f.astype(f32), 8),
        "w2": _chunk(w2f.astype(f32), 4),
        "w3": _chunk(w3f.astype(f32), 2),
        "wo": wof.astype(f32).astype(bf),
        "b1r": np.broadcast_to(b1f.astype(f32), (128, HID[0])).copy(),
        "b2r": np.broadcast_to(b2f.astype(f32), (128, HID[1])).copy(),
        "b3r": np.broadcast_to(b3f.astype(f32), (128, HID[2])).copy(),
    }

    # +-1 patterns: pass t feeds column (b8, t-1) with +1 and column
    # (b8, t) with -1 (pass 12 = raw probs only feeds column 11).
    wpat = np.zeros((NPASS, 128, PCOL), f32)
    for t in range(1, NPASS + 1):
        for e in range(E):
            for b8 in range(NB8):
                p = e * NB8 + b8
                wpat[t - 1, p, b8 * ST + (t - 1)] = 1.0
                if t < NPASS:
                    wpat[t - 1, p, b8 * ST + t] = -1.0
    consts["wpat"] = np.ascontiguousarray(
        wpat.transpose(1, 0, 2).reshape(128, NPASS * PCOL)).astype(bf)
    consts["identb"] = np.eye(128, dtype=f32).astype(bf)
    return consts


LAST_RESULTS = None


def _core_inputs(consts, emb_full, pred_full, c):
    import ml_dtypes
    bf = ml_dtypes.bfloat16
    bsl = slice(c * BL, (c + 1) * BL)
    m = dict(consts)
    embT = np.ascontiguousarray(
        emb_full[:, bsl, :].reshape(ROWS, D).T)          # [D, ROWS] f32
    m["emb"] = _chunk(embT, 8)
    pc = pred_full[:, bsl, :KU, :]                       # [E, 32, KU, 2]
    probs = pc[..., 0].astype(bf)
    offs_i = (pc[..., 1].astype(np.int32)
              - ST * np.arange(KU, dtype=np.int32)[None, None, :])
    # structural contract of the generator: idx = 12*k + offs, offs in [0,12)
    assert offs_i.min() >= 0 and offs_i.max() < ST, (
        "index structure violated: idx != 12*k + offs")
    offs = offs_i.astype(bf)
    def shuf(a):
        a = a.reshape(E, NBG, NB8, KU)
        return np.ascontiguousarray(
            a.transpose(1, 0, 2, 3).reshape(NBG, 128, KU))
    m["probs"] = shuf(probs)
    m["offs"] = shuf(offs)
    return m


def kernel(**inputs) -> np.ndarray:
    from concourse.bass_utils import run_bass_kernel_spmd

    inputs = {k: np.asarray(v) for k, v in inputs.items()}
    consts = _host_prep(inputs)
    use_bias = any(
        np.abs(consts[k]).max() > 0 for k in ("b1r", "b2r", "b3r"))
    nc = _program(use_bias)

    emb_full = np.asarray(inputs["endpoint_emb"], np.float32)
    pred_full = np.asarray(inputs["prediction"], np.float32)

    in_maps = [_core_inputs(consts, emb_full, pred_full, c)
               for c in range(NCORES)]

    res = run_bass_kernel_spmd(nc, in_maps, core_ids=list(range(NCORES)))
    global LAST_RESULTS
    LAST_RESULTS = res

    outf = np.zeros((B, V + 1, 2), np.float32)
    outf[:, :V, 1] = np.arange(V, dtype=np.float32)
    outf[:, V, 1] = -1.0
    for c in range(NCORES):
        # device out: [bg, kt, (b8*12+j), kk] -> [b, (kt,kk,j)]
        o = np.asarray(res.results[c]["out"], np.float32)
        o = o.reshape(NBG, NKT, NB8, ST, KT)
        o = o.transpose(0, 2, 1, 4, 3).reshape(BL, VU)
        outf[c * BL:(c + 1) * BL, :VU, 0] = o
    return outf
